# revision 39
# baseline (speedup 1.0000x reference)
"""Trainium2 kernel for nn_BasicBlock_53171695125036 (gnn_message_passing).

Split of work:
  - The two SubMConv3d sparse convolutions (the dominant FLOPs) run on all 8
    NeuronCores as edge-list GEMM + on-device scatter-add Bass kernels.
    The edge plan (which neighbor feeds which output row through which tap)
    is compile-time data derived from `indices`, so the HOST packs the
    gathered neighbor feature columns [C, gnc] per core (input marshalling,
    like the transposed self-feature tile) and the device loads them with
    plain HWDGE DMAs — this removes the per-launch SWDGE gather whose
    994 ns/instruction descriptor generation serialized on the gpsimd
    engine ahead of the scatter ucode in the previous design.
  - Device per launch: 3 packed input DMAs (each HWDGE DMA costs ~630 ns on
    the shared HWDGE engine + ~900 ns completion-sem propagation, and all
    transfers serialize on the shared DMA engines, so few purpose-ordered
    buffers win) -> per-tap matmuls with weights stationary (lhsT)
    producing [out_ch, edge] products in PSUM -> psum-block copies to a
    bf16 stream -> gpsimd InstScatterAdd ucode accumulates the stream into
    a zeroed accumulator [C, ROWS+8, 2] in two chunks -> DMA out.  The
    center tap (every point itself) goes through its own psum->sbuf tile
    and is summed with the scatter accumulator on host, keeping the self
    products (and their DMA) off the scatter chain.
  - Rows are assigned to cores by a greedy balancer that equalizes per-tap
    edge counts across cores (the SPMD layout pads every tap span to the
    worst core, so balancing shrinks the padded stream to its floor —
    gnc 2016 -> 1760 here); the host un-permutes the output for free.
  - The irregular per-point pipeline (CMPFE MLPs, integer kNN selection,
    voxel clustering, segment softmax aggregation) is computed on host in
    fp32, bit-faithful to the jax reference where it is discretely
    sensitive (cluster ids, kNN sets).
  - BatchNorm between the two convs needs global batch stats, so the convs
    are two launches of ONE compiled program with host stat combination
    in between. (Fusing both convs into one launch would need on-device
    global BN stats = cross-core exchange; collectives cost a flat 15 us
    in the cost model and remote_dma is unmodeled in no_exec TimelineSim,
    so the two-launch structure stands.)

Hardware facts established by experiment (axon-tunneled trn2):
  - dma_scatter_add (DMA engines) loses concurrent read-modify-writes when
    one instruction carries duplicate destination rows; adds ACROSS
    serialized instructions are exact. Unusable for this conv (every dst
    row receives ~3.3 tap contributions).
  - the InstScatterAdd gpsimd ucode processes indices in 32-wide vector
    batches: duplicate dsts >= 32 slots apart accumulate exactly, closer
    ones collapse. The edge plan guarantees the spacing (unique ascending
    dsts within a tap, >= 32-slot tap spans, pad slots aimed at a dump
    row); a checker widens spans if any input ever violates it. CoreSim's
    Python interp of this op uses buffered fancy-index += (duplicates
    collapse), so CoreSim under-reports accuracy here — device is truth.
  - raw Bass needs mybir.codegen_inst_isa_subclasses() before the NEFF
    compile and one sync-wait per instruction (_split_multiwait), with the
    hoisted-wait NOPs registered in nc.inst_map for the race detector.
"""

import os
import sys

import numpy as np

for _p in ("/opt/trn_rl_repo",):
    if _p not in sys.path and os.path.isdir(_p):
        sys.path.insert(0, _p)

N = 6144
C = 96
B = 2
D = H = W = 32
K = 16
DEPTH = 4
NCORES = 8
ROWS = N // NCORES  # 768
GRID_OPTS = np.array([[0.1, 0.1, 0.1], [0.4, 0.4, 0.4], [0.2, 0.2, 0.2]], dtype=np.float32)
BN_EPS = 1e-5

F32 = np.float32


def _bn(x, g, b):
    m = x.mean(0)
    v = x.var(0)
    return (x - m) * (1.0 / np.sqrt(v + F32(BN_EPS))) * g + b


def _relu(x):
    return np.maximum(x, F32(0.0))


def _sigmoid(x):
    return F32(1.0) / (F32(1.0) + np.exp(-x))


def _softmax(x, axis):
    e = np.exp(x - x.max(axis=axis, keepdims=True))
    return e / e.sum(axis=axis, keepdims=True)


def _seg_sum(x, seg):
    out = np.zeros((N, x.shape[1]), dtype=x.dtype)
    np.add.at(out, seg, x)
    return out


def _knn_idx(coord_i, batch):
    """Exact mirror of the reference top-k: all d2 values are small ints,
    exact in fp32, so selection == ascending (d2, index) lexicographic."""
    sq = (coord_i * coord_i).sum(1)  # int64
    d2 = sq[:, None] + sq[None, :] - 2 * (coord_i @ coord_i.T)
    same = batch[None, :] == batch[:, None]
    np.fill_diagonal(same, False)
    BIG = np.int64(1 << 40)
    key = d2 * 8192 + np.arange(N, dtype=np.int64)[None, :]
    key = np.where(same, key, BIG)
    part = np.argpartition(key, K, axis=1)[:, :K]
    pk = np.take_along_axis(key, part, axis=1)
    srt = np.argsort(pk, axis=1)
    return np.take_along_axis(part, srt, axis=1)  # [N, K]


def _host_pre(x, indices, fp_w, fp_b, fp_g, fp_be, att_w1, att_b1, att_w2, att_b2,
              ff_w1, ff_b1, ff_g, ff_be, ff_w2, ff_b2, sa_w1, sa_b1, sa_w2, sa_b2,
              fj_w1, fj_b1, fj_g, fj_be, fj_w2, fj_b2,
              proj_w, proj_g, proj_be, lw_w, lw_g, lw_be, w_w, adp_w,
              fuse_w, fuse_g, fuse_be):
    # ---- CMPFE ----
    p = _relu(_bn(x @ fp_w.T + fp_b, fp_g, fp_be))
    cd, cl, nm = p[:, :3], p[:, 3:6], p[:, 6:9]

    def _att(f, i):
        h = _relu(f @ att_w1[i].T + att_b1[i])
        return _sigmoid(h @ att_w2[i].T + att_b2[i])

    enh = np.concatenate([cd, cl * _att(cl, 0), nm * _att(nm, 1)], axis=1)
    fu = _relu(_bn(enh @ ff_w1.T + ff_b1, ff_g, ff_be)) @ ff_w2.T + ff_b2
    sem = _sigmoid(_relu(fu @ sa_w1.T + sa_b1) @ sa_w2.T + sa_b2)
    feat = fu * sem + x * (F32(1.0) - sem)

    # ---- PFAS geometry ----
    coord_i = indices[:, 1:].astype(np.int64)
    coord = indices[:, 1:].astype(F32)
    batch = indices[:, 0]
    idx = _knn_idx(coord_i, batch)
    nbr = coord[idx]  # [N, K, 3]
    cent = nbr - nbr.mean(axis=1, keepdims=True)
    cov = np.einsum('nkd,nke->nde', cent, cent) / F32(K - 1)
    S = np.linalg.svd(cov, compute_uv=False)
    Sn = S / (S.sum(axis=1, keepdims=True) + F32(1e-6))
    linearity = Sn[:, 0:1] - (Sn[:, 1] + Sn[:, 2])[:, None]
    diff = coord[:, None, :] - nbr  # [N,K,3]
    d2f = (diff * diff).sum(-1)
    nd = np.sqrt(np.maximum(d2f, F32(1e-12)))
    mean_dist = nd.mean(axis=1, keepdims=True)
    density = F32(1.0) / (mean_dist + F32(1e-6))
    fl = _relu(_bn(feat @ fj_w1.T + fj_b1, fj_g, fj_be)) @ fj_w2.T + fj_b2
    fp_ = _softmax(fl, axis=1)
    tower = (density * 2.0 + fp_[:, 0:1]) / 3.0
    backg = (np.maximum(F32(1.0) - linearity, F32(1.0) - density) + fp_[:, 1:2]) / 3.0
    line = (linearity * 2.0 + fp_[:, 2:3]) / 3.0
    lg = GRID_OPTS[2] * np.array([1.0, 1.0, 5.0], F32)
    grid_sizes = (tower * GRID_OPTS[0] + backg * GRID_OPTS[1] + line * lg + F32(1e-6)).astype(F32)

    gm = grid_sizes.mean(axis=1)
    order = np.argsort(gm, kind='stable')
    reps = [grid_sizes[order[100:200]].mean(0),
            grid_sizes[order[::-1][:100]].mean(0),
            grid_sizes[order[:100]].mean(0)]

    start = coord.min(axis=0)

    def _cluster(size):
        size = np.clip(size, F32(1e-6), None).astype(F32)
        c = np.clip(np.floor((coord - start) / size).astype(np.int64), 0, 4095)
        mx = c.max(axis=0) + 1
        ids = ((batch.astype(np.int64) * mx[0] + c[:, 0]) * mx[1] + c[:, 1]) * mx[2] + c[:, 2]
        _, inv = np.unique(ids, return_inverse=True)
        return inv.reshape(-1)

    branch_feats = []
    for i in range(DEPTH - 1):
        seg = _cluster(reps[i])
        cnt = np.maximum(_seg_sum(np.ones((N, 1), feat.dtype), seg), F32(1.0))
        pw = _relu(_bn(feat @ lw_w[i].T, lw_g[i], lw_be[i]))
        pw = pw - (_seg_sum(pw, seg) / cnt)[seg]
        pw = pw @ w_w[i].T
        pw = np.exp(pw - pw.max())
        pw = pw / (_seg_sum(pw, seg)[seg] + F32(1e-6))
        pf = _relu(_bn(feat @ proj_w[i].T, proj_g[i], proj_be[i])) * pw
        branch_feats.append(_seg_sum(pf, seg)[seg])
    adp = _softmax(feat @ adp_w.T, axis=1)
    agg = np.einsum('nc,ncd->nd', adp, np.stack(branch_feats, 1))
    last = _relu(_bn(feat @ proj_w[-1].T, proj_g[-1], proj_be[-1]))
    fused = _relu(_bn(np.concatenate([last, agg], 1) @ fuse_w.T, fuse_g, fuse_be)) + feat
    return fused.astype(F32)


def _build_gather(indices):
    """[N, 27] int64 gather map for 3x3x3 SAME conv; -1 == inactive site."""
    lut = -np.ones((B, D + 2, H + 2, W + 2), dtype=np.int64)
    bi, zi, yi, xi = indices[:, 0], indices[:, 1], indices[:, 2], indices[:, 3]
    lut[bi, zi + 1, yi + 1, xi + 1] = np.arange(N)
    gidx = np.empty((N, 27), dtype=np.int64)
    o = 0
    for dz in range(3):
        for dy in range(3):
            for dx in range(3):
                gidx[:, o] = lut[bi, zi + dz, yi + dy, xi + dx]
                o += 1
    return gidx


# ---------------- edge plan (SPMD-uniform sparse layout) ----------------
#
# The center tap (o=13) is every point itself: its features load as one
# contiguous transposed slice and its products go to a separate [C, ROWS]
# tile (identity dst order) summed with the scatter accumulator on host.
#
# The other 26 taps form a "non-center" edge stream of length gnc (multiple
# of 16): tap o occupies a fixed column span of cap[o] (max real edge count
# across cores, so the layout is SPMD-uniform). Pad slots have zero feature
# columns (host packs zeros) and dst = dump row: their products are exactly
# zero, so scatter-adding them is a no-op.
#
# Accumulation runs through the gpsimd InstScatterAdd ucode. Two measured
# hardware facts shape this:
#   * the DMA scatter-add engine loses concurrent read-modify-writes to the
#     same row (any duplicate dst in one instruction), so it is unusable for
#     this conv;
#   * the ucode scatter-add processes indices in 32-wide vector batches:
#     duplicate dsts >= 32 positions apart accumulate exactly, closer ones
#     collapse. Within a tap dsts are unique and ascending, and same-dst
#     edges of different taps sit ~cap (>= 32) positions apart; pad slots
#     point at a dummy accumulator row so they cannot collide with real
#     edges. _build_edge_plan verifies and widens caps if needed.
# The ucode layout needs an even inner dim d: the accumulator is
# [C, ROWS+8, 2] with the real value at j=0, a dead j=1 lane, and rows
# >= ROWS as the pad dump. It starts from a memset-0 tile, so the scatter
# chain never waits on the self-feature path.

_TAPS = [o for o in range(27) if o != 13]


def _balance_rows(gidx):
    """Greedy row->core assignment (equal 768-row shards) minimizing
    sum_o max_cc count(cc, o) — the padded edge-stream length is set by the
    worst core per tap, so balancing tap counts across cores shrinks gnc.
    The host un-permutes the output for free."""
    present = gidx[:, _TAPS] >= 0  # [N, 26]
    order = np.argsort(-present.sum(1), kind="stable")
    counts = np.zeros((NCORES, len(_TAPS)), dtype=np.int64)
    fill = np.zeros(NCORES, dtype=np.int64)
    rowmap = np.empty((NCORES, ROWS), dtype=np.int64)
    for r in order:
        s = present[r]
        cur_max = counts.max(axis=0)
        best, best_cost = None, None
        for cc in range(NCORES):
            if fill[cc] >= ROWS:
                continue
            # increase in sum-of-maxes if row r goes to core cc
            cost = (np.maximum(counts[cc] + s, cur_max).sum(), counts[cc][s].sum(), fill[cc])
            if best is None or cost < best_cost:
                best, best_cost = cc, cost
        counts[best] += s
        rowmap[best, fill[best]] = r
        fill[best] += 1
    assert (fill == ROWS).all()
    return rowmap


def _wrap16(a):
    """[L] -> [128, L//16] wrapped index layout (idx i at partition
    i%16, col i//16, replicated to the 8 gpsimd cores)."""
    w = a.reshape(-1, 16).T  # [16, L//16]
    return np.ascontiguousarray(np.tile(w, (8, 1)).astype(np.int16))


def _build_edge_plan(indices, uchunks_spec=(784,), psum_block=392,
                     icuts=(512, 880), queues="SAA", warm_pre=0,
                     bridges=(), copy_split=1, copy_eng="dve", balance=True):
    gidx = _build_gather(indices)  # [N, 27], -1 invalid
    if balance:
        rowmap = _balance_rows(gidx)
    else:
        rowmap = np.arange(N, dtype=np.int64).reshape(NCORES, ROWS)
    counts = np.zeros(27, dtype=np.int64)
    for o in _TAPS:
        v = gidx[:, o] >= 0
        counts[o] = max(v[rowmap[c]].sum() for c in range(NCORES))
    # caps need no alignment (matmul spans and idx values are arbitrary;
    # only chunk boundaries are 16-aligned) — but >= 32 when non-empty so
    # same-dst edges of neighboring taps stay >= 32 apart for the ucode
    caps = {o: int(max(counts[o], 32)) if counts[o] else 0 for o in _TAPS}

    def _layout(caps):
        gnc = sum(caps.values())
        caps = dict(caps)
        pad = (-gnc) % 16  # idx wrap needs a multiple of 16
        for o in reversed(_TAPS):
            if caps[o] > 0 or o == _TAPS[-1]:
                caps[o] += pad
                break
        gnc += pad
        offs = {}
        cur = 0
        for o in _TAPS:
            offs[o] = cur
            cur += caps[o]
        assert cur == gnc
        # per-core index streams (non-center only)
        gsrc = np.full((NCORES, gnc), N, dtype=np.int64)    # pad -> zero col
        sdst = np.full((NCORES, gnc), ROWS, dtype=np.int64)  # pad -> dump row
        for cc in range(NCORES):
            g = gidx[rowmap[cc]]
            for o in _TAPS:
                if caps[o] == 0:
                    continue
                v = np.nonzero(g[:, o] >= 0)[0]
                gsrc[cc, offs[o]:offs[o] + len(v)] = g[v, o]
                sdst[cc, offs[o]:offs[o] + len(v)] = v
        return caps, gnc, offs, gsrc, sdst

    # the ucode scatter-add collapses duplicate dsts closer than 32 slots in
    # one call: widen the earlier tap's span until no real-real pair violates
    for _ in range(32):
        caps2, gnc, offs, gsrc, sdst = _layout(caps)
        bad_tap = None
        for cc in range(NCORES):
            d, real = sdst[cc], sdst[cc] < ROWS
            for w in range(1, 32):
                m = np.nonzero((d[:-w] == d[w:]) & real[:-w] & real[w:])[0]
                if len(m):
                    p = int(m[0])
                    for o in _TAPS:
                        if caps2[o] and offs[o] <= p < offs[o] + caps2[o]:
                            bad_tap = o
                            break
                    break
            if bad_tap is not None:
                break
        if bad_tap is None:
            break
        caps[bad_tap] += 32
    else:
        raise RuntimeError("could not satisfy scatter-add min-distance")
    caps = caps2

    # ucode scatter chunks (multiples of 16): per-call cost is
    # max(accumulator_free, 2*chunk) * 1.39ns + 95ns. uchunks_spec gives the
    # boundaries of all but the last chunk.
    uchunks = []
    p = 0
    for b in uchunks_spec:
        b = min(b, gnc)
        if b > p:
            uchunks.append((p, b))
            p = b
    if p < gnc:
        uchunks.append((p, gnc))

    # The inputs ride in a few packed DMA buffers (every HWDGE DMA costs
    # ~630ns on the shared HWDGE engine plus a 900ns completion-semaphore
    # propagation, and all transfers serialize on the shared DMA engines —
    # so few, purpose-ordered DMAs win). Buffer i carries the weights of the
    # taps whose span STARTS in its edge-column range (so every matmul
    # segment's weights arrive no later than its edge columns), followed by
    # those edge columns; the last buffer also carries the center-tap
    # weights and the transposed self features.
    cuts = [c for c in icuts if c < gnc] + [gnc]
    bufspec = []
    p = 0
    for bi, c1 in enumerate(cuts):
        taps = [o for o in _TAPS if caps[o] and p <= offs[o] < c1]
        bufspec.append(dict(c0=p, c1=c1, taps=taps, has_self=False))
        p = c1
    bufspec[-1]["taps"].append(13)
    bufspec[-1]["has_self"] = True
    gchunks = [(b["c0"], b["c1"]) for b in bufspec]

    # matmul segments per psum_block of the edge stream: (block, c0, c1, tap)
    # (c0/c1 are block-local columns; products go to PSUM columns, so no
    # partition-alignment constraints).
    def _clip_spans(spans, total, extra_cuts=()):
        cuts = sorted(set(range(0, total + psum_block, psum_block))
                      | set(extra_cuts))
        nblk = -(-total // psum_block)
        out = [[] for _ in range(nblk)]
        for a, bnd, o in spans:
            p = a
            while p < bnd:
                lim = min(bnd, min(c for c in cuts if c > p))
                blk = p // psum_block
                out[blk].append((p - blk * psum_block, lim - blk * psum_block, o))
                p = lim
        return out

    segs_nc = _clip_spans(
        [(offs[o], offs[o] + caps[o], o) for o in _TAPS if caps[o] > 0], gnc,
        extra_cuts=[c0 for (c0, _c1) in gchunks])
    segs_c = _clip_spans([(0, ROWS, 13)], ROWS)

    ixs = np.concatenate([_wrap16(sdst[cc])[None] for cc in range(NCORES)],
                         axis=0)  # [NCORES, 128, gnc//16]

    # tap -> (buffer id, weight column offset within buffer); buffer widths
    wcol = {}
    for bi, b in enumerate(bufspec):
        for i, o in enumerate(b["taps"]):
            wcol[o] = (bi, i * C)
        b["nw"] = len(b["taps"]) * C
        b["W"] = b["nw"] + (b["c1"] - b["c0"]) + (ROWS if b["has_self"] else 0)

    return dict(gnc=gnc, segs_nc=segs_nc, segs_c=segs_c, gchunks=gchunks,
                uchunks=uchunks, psum_block=psum_block, ixs=ixs, gsrc=gsrc,
                rowmap=rowmap,
                bufspec=bufspec, wcol=wcol, queues=queues, warm_pre=warm_pre,
                bridges=dict(bridges), copy_split=copy_split, copy_eng=copy_eng,
                sig=(gnc, tuple(caps[o] for o in _TAPS), tuple(uchunks_spec),
                     psum_block, tuple(cuts), queues, warm_pre,
                     tuple(sorted(dict(bridges).items())), copy_split, copy_eng))


# ---------------- Bass device program ----------------
_CACHED = {}


def _split_multiwait(nc):
    """Walrus encodes at most one sync wait per instruction. Hoist extra
    waits onto same-engine NOPs inserted just before."""
    import concourse.mybir as mybir

    ctr = 0
    for fn in nc.m.functions:
        for bb in fn.blocks:
            insts = bb.instructions
            orig = list(insts)
            newlist = []
            for inst in orig:
                si = inst.sync_info
                waits = list(si.on_wait or []) if si is not None else []
                if len(waits) >= 2:
                    for w in waits:
                        nop = mybir.InstNoOp(name=f"I-wsplit{ctr}", ins=[], outs=[])
                        ctr += 1
                        nop.engine = inst.engine
                        nop.sync_info = mybir.SyncInfo(on_wait=[w], on_update=[])
                        # register so CoreSim's race detector sees it (its
                        # fake-sem-update pass walks inst_map, not the blocks)
                        nc.inst_map[nop.name] = nop
                        newlist.append(nop)
                    inst.sync_info = mybir.SyncInfo(
                        on_wait=[], on_update=list(si.on_update or []))
                newlist.append(inst)
            insts.clear()
            insts.extend(newlist)


def _build_conv_program(plan):
    import concourse.bass as bass
    import concourse.mybir as mybir
    import concourse.tile as tile
    from concourse import library_config

    nc = bass.Bass("TRN2")
    f32 = mybir.dt.float32
    bf16 = mybir.dt.bfloat16
    i16 = mybir.dt.int16

    gnc = plan["gnc"]
    segs_nc = plan["segs_nc"]
    segs_c = plan["segs_c"]
    uchunks = plan["uchunks"]
    pb = plan["psum_block"]
    wcol = plan["wcol"]
    bufspec = plan["bufspec"]
    queues = plan["queues"]
    warm_pre = plan["warm_pre"]
    bridges = plan["bridges"]
    copy_split = plan["copy_split"]
    copy_eng = plan["copy_eng"]
    ixw = gnc // 16

    bufds = [nc.dram_tensor(f"buf{bi}", [C, b["W"]], bf16, kind="ExternalInput")
             for bi, b in enumerate(bufspec)]
    ixs = nc.dram_tensor("ixs", [128, ixw], i16, kind="ExternalInput")
    Y = nc.dram_tensor("Y", [C, ROWS * 2], bf16, kind="ExternalOutput")
    CEN = nc.dram_tensor("CEN", [C, ROWS], bf16, kind="ExternalOutput")

    from contextlib import ExitStack
    with ExitStack() as ctx:
        tc = ctx.enter_context(
            tile.TileContext(nc, linearize=os.environ.get("KERNEL_LINEARIZE", "0") == "1"))
        const = ctx.enter_context(tc.tile_pool(name="const", bufs=1))
        psum_pool = ctx.enter_context(tc.tile_pool(name="pp", bufs=4, space="PSUM"))
        dmy_pool = ctx.enter_context(tc.tile_pool(name="dmy", bufs=1, space="PSUM"))
        scat_pool = ctx.enter_context(tc.tile_pool(name="scat", bufs=1))

        nc.gpsimd.load_library(library_config.mlp)

        # DMAs in. Only SP and Activation have HWDGE queues; gpsimd can also
        # start (SWDGE) DMAs and is idle until the first scatter. The packed
        # buffers ride the queues given by plan["queues"]; the scatter idxs
        # ride the gpsimd SWDGE queue (they only gate the ucode chain).
        # PE p-state warm-up (optional; off by default — the cost model runs
        # matmuls at 1.2 GHz until 3us of continuous PE busy, but dummy-chain
        # warming measured net-neutral to negative here since the matmuls are
        # off the critical path).
        if warm_pre or bridges:
            dmy = const.tile([C, 64], bf16)
            nc.scalar.memzero(dmy[:])
            pd = dmy_pool.tile([64, 64], f32)

        def emit_warm(n):
            for _ in range(n):
                nc.tensor.matmul(pd[0:64, :], lhsT=dmy[0:C, 0:64],
                                 rhs=dmy[0:C, 0:64],
                                 start=True, stop=True, skip_group_check=True)

        # accumulator [C, ROWS+8, 2] bf16 (j=0 real, j=1 dead lane for d=2,
        # rows >= ROWS take the pad-slot adds), zeroed up front so the
        # scatter chain is independent of the center/self path. The j=1
        # lanes and dump rows accumulate garbage; the host only reads j=0 of
        # rows < ROWS. scat j=1 lanes are never written (uninitialized),
        # matching the accumulator's dead lanes.
        ysb = scat_pool.tile([C, ROWS + 8, 2], bf16)
        nc.vector.memset(ysb[:], 0.0)
        scat = scat_pool.tile([C, gnc, 2], bf16)
        cen = scat_pool.tile([C, ROWS], bf16)

        ixb = const.tile([128, ixw], i16)
        nc.gpsimd.dma_start(ixb[:], ixs[:])
        qmap = {"S": nc.sync, "A": nc.scalar}
        bufs = []
        for bi, b in enumerate(bufspec):
            t = const.tile([C, b["W"]], bf16, tag=f"buf{bi}")
            qmap[queues[bi]].dma_start(t[:], bufds[bi][:])
            bufs.append(t)

        emit_warm(warm_pre)

        def wslice(o):
            bi, off = wcol[o]
            return bufs[bi][0:C, off:off + C]

        def rhs_slice(col, ln):
            for bi, b in enumerate(bufspec):
                if b["c0"] <= col < b["c1"]:
                    off = b["nw"] + col - b["c0"]
                    return bufs[bi][0:C, off:off + ln]
            raise AssertionError(col)

        def self_slice(col, ln):
            b = bufspec[-1]
            off = b["nw"] + (b["c1"] - b["c0"])
            return bufs[-1][0:C, off + col:off + col + ln]

        # products: out[oc, edge] = sum_ic W[ic, o, oc] * feat[ic, edge]
        # (weights stationary, feature columns moving -> arbitrary column
        # spans, no PSUM partition-alignment constraints)
        ncopies = 0

        def emit_copy(eng, dst, src):
            if eng is nc.scalar:
                eng.activation(dst, src, mybir.ActivationFunctionType.Copy)
            else:
                eng.tensor_copy(dst, src)

        def emit_block(blk, seglist, lhs_fn, out_tile):
            nonlocal ncopies
            lo = blk * pb
            width = max(c1 for (c0, c1, _o) in seglist)
            pp = psum_pool.tile([C, pb], f32, tag="pp")
            for (c0, c1, o) in seglist:
                nc.tensor.matmul(
                    pp[0:C, c0:c1],
                    lhsT=wslice(o),
                    rhs=lhs_fn(lo + c0, c1 - c0),
                    start=True, stop=True, skip_group_check=True)
            # copy psum -> bf16, split into pieces across copy engines so
            # the scatter's gate closes sooner. In "dvp" mode the blocks of
            # the FIRST scatter chunk split DVE+gpsimd: the gpsimd halves
            # precede the first scatter in its own engine order (no extra
            # semaphore hop), and gpsimd is idle until then anyway. Later
            # blocks stay off gpsimd so they don't wedge between scatters.
            dvp = (copy_eng == "dvp" and out_tile is scat
                   and lo + width <= uchunks[0][1])
            pieces = 2 if dvp else max(1, min(copy_split, width // 128))
            step = -(-width // pieces)
            for pi in range(pieces):
                a, bnd = pi * step, min((pi + 1) * step, width)
                if dvp:
                    eng = nc.vector if pi == 0 else nc.gpsimd
                elif copy_eng in ("dve", "dvp"):
                    eng = nc.vector
                elif copy_eng == "act":
                    eng = nc.scalar
                else:
                    eng = nc.vector if (ncopies % 2 == 0) else nc.scalar
                if out_tile is cen:
                    dst = out_tile[:, lo + a:lo + bnd]
                else:
                    dst = out_tile[:, lo + a:lo + bnd, 0]
                emit_copy(eng, dst, pp[0:C, a:bnd])
                ncopies += 1

        def ucode_of(col):
            for ui, (u0, u1) in enumerate(uchunks):
                if u0 <= col < u1:
                    return ui
            raise AssertionError(col)

        # non-center blocks fill the ucode add stream; alternate copy
        # engines within each ucode chunk
        emitted_isa = set()
        for blk, seglist in enumerate(segs_nc):
            emit_block(blk, seglist, rhs_slice, scat)
            # keep the PE hot across data-arrival gaps
            emit_warm(bridges.get(blk, 0))
            # emit each scatter chunk as soon as its blocks are done so the
            # Pool SEQ order matches data availability
            nxt = (blk + 1) * pb
            for ui, (u0, u1) in enumerate(uchunks):
                if ui not in emitted_isa and nxt >= u1:
                    nc.gpsimd.scatter_add(
                        ysb[:], ixb[0:C, u0 // 16:u1 // 16],
                        scat[:, u0:u1, :], C, ROWS + 8, 2, u1 - u0)
                    emitted_isa.add(ui)
        assert emitted_isa == set(range(len(uchunks)))

        # center blocks go to their own tile (host adds it to the scatter
        # accumulator), keeping the self path off the ucode chain
        for blk, seglist in enumerate(segs_c):
            emit_block(blk, seglist, self_slice, cen)

        nc.scalar.dma_start(CEN[:], cen[:])
        nc.sync.dma_start(Y[:], ysb[:, 0:ROWS, :])
    if os.environ.get("KERNEL_SPLIT_MULTIWAIT", "1") == "1":
        _split_multiwait(nc)
    # Raw Bass skips Bacc's codegen_inst_isa_subclasses pass; without it the
    # NEFF compiler sees empty .instr bytes for extended-ISA instructions
    # (e.g. the library reload) and fails with "ISA wrong length".
    mybir.codegen_inst_isa_subclasses(nc)
    return nc


def _run_conv(feats_f32, plan, w_flat):
    """feats_f32 [N, C] f32, w_flat [27, C, C] f32 -> conv output [N, C] f32
    (no bias; SubMConv3d has none)."""
    from concourse.bass_utils import run_bass_kernel_spmd

    if _CACHED.get("sig") != plan["sig"]:
        _CACHED["nc"] = _build_conv_program(plan)
        _CACHED["sig"] = plan["sig"]
    nc = _CACHED["nc"]

    import ml_dtypes
    fpb = np.ascontiguousarray(feats_f32).astype(ml_dtypes.bfloat16)
    wb = np.ascontiguousarray(np.transpose(w_flat, (1, 0, 2)))  # [ic, o, oc]
    wbb = wb.astype(ml_dtypes.bfloat16)

    gnc = plan["gnc"]
    gsrc = plan["gsrc"]
    in_maps = []
    for cc in range(NCORES):
        edgeT = np.zeros((C, gnc), dtype=ml_dtypes.bfloat16)
        real = gsrc[cc] < N
        edgeT[:, real] = fpb[gsrc[cc][real]].T
        m = {"ixs": plan["ixs"][cc]}
        for bi, b in enumerate(plan["bufspec"]):
            parts = [wbb[:, o, :] for o in b["taps"]]
            parts.append(edgeT[:, b["c0"]:b["c1"]])
            if b["has_self"]:
                parts.append(fpb[plan["rowmap"][cc], :].T)
            buf = np.concatenate(parts, axis=1)
            assert buf.shape[1] == b["W"]
            m[f"buf{bi}"] = np.ascontiguousarray(buf)
        in_maps.append(m)
    trace = os.environ.get("KERNEL_TRACE", "") == "1"
    res = run_bass_kernel_spmd(nc, in_maps, core_ids=list(range(NCORES)), trace=trace)
    if trace and res.exec_time_ns is not None:
        print(f"HW exec time: {res.exec_time_ns} ns")
        _CACHED.setdefault("exec_ns", []).append(res.exec_time_ns)
    out = np.empty((N, C), dtype=np.float32)
    for cc in range(NCORES):
        Yc = np.asarray(res.results[cc]["Y"]).astype(np.float32).reshape(C, ROWS, 2)
        Cc = np.asarray(res.results[cc]["CEN"]).astype(np.float32)
        out[plan["rowmap"][cc]] = (Yc[:, :, 0] + Cc).T
    return out


def _conv_host(feats_f32, plan, w_flat):
    """Host fallback/validation path for the conv (numpy, fp32)."""
    del plan
    gidx = _build_gather(_CACHED["indices"])
    acc = np.zeros((N, C), dtype=np.float32)
    for o in range(27):
        v = gidx[:, o] >= 0
        acc[v] += feats_f32[gidx[v, o]] @ w_flat[o]
    return acc


def kernel(**inputs):
    inputs = {k: np.asarray(v) for k, v in inputs.items()}
    fused = _host_pre(
        inputs['x'], inputs['indices'], inputs['fp_w'], inputs['fp_b'], inputs['fp_g'],
        inputs['fp_be'], inputs['att_w1'], inputs['att_b1'], inputs['att_w2'], inputs['att_b2'],
        inputs['ff_w1'], inputs['ff_b1'], inputs['ff_g'], inputs['ff_be'], inputs['ff_w2'],
        inputs['ff_b2'], inputs['sa_w1'], inputs['sa_b1'], inputs['sa_w2'], inputs['sa_b2'],
        inputs['fj_w1'], inputs['fj_b1'], inputs['fj_g'], inputs['fj_be'], inputs['fj_w2'],
        inputs['fj_b2'], inputs['proj_w'], inputs['proj_g'], inputs['proj_be'], inputs['lw_w'],
        inputs['lw_g'], inputs['lw_be'], inputs['w_w'], inputs['adp_w'], inputs['fuse_w'],
        inputs['fuse_g'], inputs['fuse_be'])

    _CACHED["indices"] = inputs['indices']
    key = inputs['indices'].tobytes()
    if _CACHED.get("plan_key") != key:
        _CACHED["plan"] = _build_edge_plan(inputs['indices'])
        _CACHED["plan_key"] = key
    plan = _CACHED["plan"]

    w1 = inputs['conv1_w'].reshape(27, C, C).astype(np.float32)
    w2 = inputs['conv2_w'].reshape(27, C, C).astype(np.float32)

    conv = _conv_host if os.environ.get("KERNEL_HOST_CONV", "") == "1" else _run_conv

    raw1 = conv(fused, plan, w1)
    f1 = _relu(_bn(raw1, inputs['bn1_g'], inputs['bn1_be']))
    raw2 = conv(f1, plan, w2)
    f2 = _bn(raw2, inputs['bn2_g'], inputs['bn2_be'])
    return _relu(f2 + fused).astype(np.float32)


# revision 41
# speedup vs baseline: 1.0046x; 1.0046x over previous
"""Trainium2 kernel for nn_BasicBlock_53171695125036 (gnn_message_passing).

Split of work:
  - The two SubMConv3d sparse convolutions (the dominant FLOPs) run on all 8
    NeuronCores as edge-list GEMM + on-device scatter-add Bass kernels.
    The edge plan (which neighbor feeds which output row through which tap)
    is compile-time data derived from `indices`, so the HOST packs the
    gathered neighbor feature columns [C, gnc] per core (input marshalling,
    like the transposed self-feature tile) and the device loads them with
    plain HWDGE DMAs — this removes the per-launch SWDGE gather whose
    994 ns/instruction descriptor generation serialized on the gpsimd
    engine ahead of the scatter ucode in the previous design.
  - Device per launch: 3 packed input DMAs (each HWDGE DMA costs ~630 ns on
    the shared HWDGE engine + ~900 ns completion-sem propagation, and all
    transfers serialize on the shared DMA engines, so few purpose-ordered
    buffers win) -> per-tap matmuls with weights stationary (lhsT)
    producing [out_ch, edge] products in PSUM -> psum-block copies to a
    bf16 stream -> gpsimd InstScatterAdd ucode accumulates the stream into
    a zeroed accumulator [C, ROWS+8, 2] in two chunks -> DMA out.  The
    center tap (every point itself) goes through its own psum->sbuf tile
    and is summed with the scatter accumulator on host, keeping the self
    products (and their DMA) off the scatter chain.
  - Rows are assigned to cores by a greedy balancer that equalizes per-tap
    edge counts across cores (the SPMD layout pads every tap span to the
    worst core, so balancing shrinks the padded stream to its floor —
    gnc 2016 -> 1760 here); the host un-permutes the output for free.
  - The irregular per-point pipeline (CMPFE MLPs, integer kNN selection,
    voxel clustering, segment softmax aggregation) is computed on host in
    fp32, bit-faithful to the jax reference where it is discretely
    sensitive (cluster ids, kNN sets).
  - BatchNorm between the two convs needs global batch stats, so the convs
    are two launches of ONE compiled program with host stat combination
    in between. (Fusing both convs into one launch would need on-device
    global BN stats = cross-core exchange; collectives cost a flat 15 us
    in the cost model and remote_dma is unmodeled in no_exec TimelineSim,
    so the two-launch structure stands.)

Hardware facts established by experiment (axon-tunneled trn2):
  - dma_scatter_add (DMA engines) loses concurrent read-modify-writes when
    one instruction carries duplicate destination rows; adds ACROSS
    serialized instructions are exact. Unusable for this conv (every dst
    row receives ~3.3 tap contributions).
  - the InstScatterAdd gpsimd ucode processes indices in 32-wide vector
    batches: duplicate dsts >= 32 slots apart accumulate exactly, closer
    ones collapse. The edge plan guarantees the spacing (unique ascending
    dsts within a tap, >= 32-slot tap spans, pad slots aimed at a dump
    row); a checker widens spans if any input ever violates it. CoreSim's
    Python interp of this op uses buffered fancy-index += (duplicates
    collapse), so CoreSim under-reports accuracy here — device is truth.
  - raw Bass needs mybir.codegen_inst_isa_subclasses() before the NEFF
    compile and one sync-wait per instruction (_split_multiwait), with the
    hoisted-wait NOPs registered in nc.inst_map for the race detector.
"""

import os
import sys

import numpy as np

for _p in ("/opt/trn_rl_repo",):
    if _p not in sys.path and os.path.isdir(_p):
        sys.path.insert(0, _p)

N = 6144
C = 96
B = 2
D = H = W = 32
K = 16
DEPTH = 4
NCORES = 8
ROWS = N // NCORES  # 768
GRID_OPTS = np.array([[0.1, 0.1, 0.1], [0.4, 0.4, 0.4], [0.2, 0.2, 0.2]], dtype=np.float32)
BN_EPS = 1e-5

F32 = np.float32


def _bn(x, g, b):
    m = x.mean(0)
    v = x.var(0)
    return (x - m) * (1.0 / np.sqrt(v + F32(BN_EPS))) * g + b


def _relu(x):
    return np.maximum(x, F32(0.0))


def _sigmoid(x):
    return F32(1.0) / (F32(1.0) + np.exp(-x))


def _softmax(x, axis):
    e = np.exp(x - x.max(axis=axis, keepdims=True))
    return e / e.sum(axis=axis, keepdims=True)


def _seg_sum(x, seg):
    out = np.zeros((N, x.shape[1]), dtype=x.dtype)
    np.add.at(out, seg, x)
    return out


def _knn_idx(coord_i, batch):
    """Exact mirror of the reference top-k: all d2 values are small ints,
    exact in fp32, so selection == ascending (d2, index) lexicographic."""
    sq = (coord_i * coord_i).sum(1)  # int64
    d2 = sq[:, None] + sq[None, :] - 2 * (coord_i @ coord_i.T)
    same = batch[None, :] == batch[:, None]
    np.fill_diagonal(same, False)
    BIG = np.int64(1 << 40)
    key = d2 * 8192 + np.arange(N, dtype=np.int64)[None, :]
    key = np.where(same, key, BIG)
    part = np.argpartition(key, K, axis=1)[:, :K]
    pk = np.take_along_axis(key, part, axis=1)
    srt = np.argsort(pk, axis=1)
    return np.take_along_axis(part, srt, axis=1)  # [N, K]


def _host_pre(x, indices, fp_w, fp_b, fp_g, fp_be, att_w1, att_b1, att_w2, att_b2,
              ff_w1, ff_b1, ff_g, ff_be, ff_w2, ff_b2, sa_w1, sa_b1, sa_w2, sa_b2,
              fj_w1, fj_b1, fj_g, fj_be, fj_w2, fj_b2,
              proj_w, proj_g, proj_be, lw_w, lw_g, lw_be, w_w, adp_w,
              fuse_w, fuse_g, fuse_be):
    # ---- CMPFE ----
    p = _relu(_bn(x @ fp_w.T + fp_b, fp_g, fp_be))
    cd, cl, nm = p[:, :3], p[:, 3:6], p[:, 6:9]

    def _att(f, i):
        h = _relu(f @ att_w1[i].T + att_b1[i])
        return _sigmoid(h @ att_w2[i].T + att_b2[i])

    enh = np.concatenate([cd, cl * _att(cl, 0), nm * _att(nm, 1)], axis=1)
    fu = _relu(_bn(enh @ ff_w1.T + ff_b1, ff_g, ff_be)) @ ff_w2.T + ff_b2
    sem = _sigmoid(_relu(fu @ sa_w1.T + sa_b1) @ sa_w2.T + sa_b2)
    feat = fu * sem + x * (F32(1.0) - sem)

    # ---- PFAS geometry ----
    coord_i = indices[:, 1:].astype(np.int64)
    coord = indices[:, 1:].astype(F32)
    batch = indices[:, 0]
    idx = _knn_idx(coord_i, batch)
    nbr = coord[idx]  # [N, K, 3]
    cent = nbr - nbr.mean(axis=1, keepdims=True)
    cov = np.einsum('nkd,nke->nde', cent, cent) / F32(K - 1)
    S = np.linalg.svd(cov, compute_uv=False)
    Sn = S / (S.sum(axis=1, keepdims=True) + F32(1e-6))
    linearity = Sn[:, 0:1] - (Sn[:, 1] + Sn[:, 2])[:, None]
    diff = coord[:, None, :] - nbr  # [N,K,3]
    d2f = (diff * diff).sum(-1)
    nd = np.sqrt(np.maximum(d2f, F32(1e-12)))
    mean_dist = nd.mean(axis=1, keepdims=True)
    density = F32(1.0) / (mean_dist + F32(1e-6))
    fl = _relu(_bn(feat @ fj_w1.T + fj_b1, fj_g, fj_be)) @ fj_w2.T + fj_b2
    fp_ = _softmax(fl, axis=1)
    tower = (density * 2.0 + fp_[:, 0:1]) / 3.0
    backg = (np.maximum(F32(1.0) - linearity, F32(1.0) - density) + fp_[:, 1:2]) / 3.0
    line = (linearity * 2.0 + fp_[:, 2:3]) / 3.0
    lg = GRID_OPTS[2] * np.array([1.0, 1.0, 5.0], F32)
    grid_sizes = (tower * GRID_OPTS[0] + backg * GRID_OPTS[1] + line * lg + F32(1e-6)).astype(F32)

    gm = grid_sizes.mean(axis=1)
    order = np.argsort(gm, kind='stable')
    reps = [grid_sizes[order[100:200]].mean(0),
            grid_sizes[order[::-1][:100]].mean(0),
            grid_sizes[order[:100]].mean(0)]

    start = coord.min(axis=0)

    def _cluster(size):
        size = np.clip(size, F32(1e-6), None).astype(F32)
        c = np.clip(np.floor((coord - start) / size).astype(np.int64), 0, 4095)
        mx = c.max(axis=0) + 1
        ids = ((batch.astype(np.int64) * mx[0] + c[:, 0]) * mx[1] + c[:, 1]) * mx[2] + c[:, 2]
        _, inv = np.unique(ids, return_inverse=True)
        return inv.reshape(-1)

    branch_feats = []
    for i in range(DEPTH - 1):
        seg = _cluster(reps[i])
        cnt = np.maximum(_seg_sum(np.ones((N, 1), feat.dtype), seg), F32(1.0))
        pw = _relu(_bn(feat @ lw_w[i].T, lw_g[i], lw_be[i]))
        pw = pw - (_seg_sum(pw, seg) / cnt)[seg]
        pw = pw @ w_w[i].T
        pw = np.exp(pw - pw.max())
        pw = pw / (_seg_sum(pw, seg)[seg] + F32(1e-6))
        pf = _relu(_bn(feat @ proj_w[i].T, proj_g[i], proj_be[i])) * pw
        branch_feats.append(_seg_sum(pf, seg)[seg])
    adp = _softmax(feat @ adp_w.T, axis=1)
    agg = np.einsum('nc,ncd->nd', adp, np.stack(branch_feats, 1))
    last = _relu(_bn(feat @ proj_w[-1].T, proj_g[-1], proj_be[-1]))
    fused = _relu(_bn(np.concatenate([last, agg], 1) @ fuse_w.T, fuse_g, fuse_be)) + feat
    return fused.astype(F32)


def _build_gather(indices):
    """[N, 27] int64 gather map for 3x3x3 SAME conv; -1 == inactive site."""
    lut = -np.ones((B, D + 2, H + 2, W + 2), dtype=np.int64)
    bi, zi, yi, xi = indices[:, 0], indices[:, 1], indices[:, 2], indices[:, 3]
    lut[bi, zi + 1, yi + 1, xi + 1] = np.arange(N)
    gidx = np.empty((N, 27), dtype=np.int64)
    o = 0
    for dz in range(3):
        for dy in range(3):
            for dx in range(3):
                gidx[:, o] = lut[bi, zi + dz, yi + dy, xi + dx]
                o += 1
    return gidx


# ---------------- edge plan (SPMD-uniform sparse layout) ----------------
#
# The center tap (o=13) is every point itself: its features load as one
# contiguous transposed slice and its products go to a separate [C, ROWS]
# tile (identity dst order) summed with the scatter accumulator on host.
#
# The other 26 taps form a "non-center" edge stream of length gnc (multiple
# of 16): tap o occupies a fixed column span of cap[o] (max real edge count
# across cores, so the layout is SPMD-uniform). Pad slots have zero feature
# columns (host packs zeros) and dst = dump row: their products are exactly
# zero, so scatter-adding them is a no-op.
#
# Accumulation runs through the gpsimd InstScatterAdd ucode. Two measured
# hardware facts shape this:
#   * the DMA scatter-add engine loses concurrent read-modify-writes to the
#     same row (any duplicate dst in one instruction), so it is unusable for
#     this conv;
#   * the ucode scatter-add processes indices in 32-wide vector batches:
#     duplicate dsts >= 32 positions apart accumulate exactly, closer ones
#     collapse. Within a tap dsts are unique and ascending, and same-dst
#     edges of different taps sit ~cap (>= 32) positions apart; pad slots
#     point at a dummy accumulator row so they cannot collide with real
#     edges. _build_edge_plan verifies and widens caps if needed.
# The ucode layout needs an even inner dim d: the accumulator is
# [C, ROWS+8, 2] with the real value at j=0, a dead j=1 lane, and rows
# >= ROWS as the pad dump. It starts from a memset-0 tile, so the scatter
# chain never waits on the self-feature path.

_TAPS = [o for o in range(27) if o != 13]


def _balance_rows(gidx):
    """Greedy row->core assignment (equal 768-row shards) minimizing
    sum_o max_cc count(cc, o) — the padded edge-stream length is set by the
    worst core per tap, so balancing tap counts across cores shrinks gnc.
    The host un-permutes the output for free."""
    present = gidx[:, _TAPS] >= 0  # [N, 26]
    order = np.argsort(-present.sum(1), kind="stable")
    counts = np.zeros((NCORES, len(_TAPS)), dtype=np.int64)
    fill = np.zeros(NCORES, dtype=np.int64)
    rowmap = np.empty((NCORES, ROWS), dtype=np.int64)
    for r in order:
        s = present[r]
        cur_max = counts.max(axis=0)
        best, best_cost = None, None
        for cc in range(NCORES):
            if fill[cc] >= ROWS:
                continue
            # increase in sum-of-maxes if row r goes to core cc
            cost = (np.maximum(counts[cc] + s, cur_max).sum(), counts[cc][s].sum(), fill[cc])
            if best is None or cost < best_cost:
                best, best_cost = cc, cost
        counts[best] += s
        rowmap[best, fill[best]] = r
        fill[best] += 1
    assert (fill == ROWS).all()
    return rowmap


def _wrap16(a):
    """[L] -> [128, L//16] wrapped index layout (idx i at partition
    i%16, col i//16, replicated to the 8 gpsimd cores)."""
    w = a.reshape(-1, 16).T  # [16, L//16]
    return np.ascontiguousarray(np.tile(w, (8, 1)).astype(np.int16))


def _build_edge_plan(indices, uchunks_spec=(784,), psum_block=392,
                     icuts=(512, 880), queues="SAA", warm_pre=0,
                     bridges=(), copy_split=1, copy_eng="dve", balance=True):
    gidx = _build_gather(indices)  # [N, 27], -1 invalid
    if balance:
        rowmap = _balance_rows(gidx)
    else:
        rowmap = np.arange(N, dtype=np.int64).reshape(NCORES, ROWS)
    counts = np.zeros(27, dtype=np.int64)
    for o in _TAPS:
        v = gidx[:, o] >= 0
        counts[o] = max(v[rowmap[c]].sum() for c in range(NCORES))
    # caps need no alignment (matmul spans and idx values are arbitrary;
    # only chunk boundaries are 16-aligned) — but >= 32 when non-empty so
    # same-dst edges of neighboring taps stay >= 32 apart for the ucode
    caps = {o: int(max(counts[o], 32)) if counts[o] else 0 for o in _TAPS}

    def _layout(caps):
        gnc = sum(caps.values())
        caps = dict(caps)
        pad = (-gnc) % 16  # idx wrap needs a multiple of 16
        for o in reversed(_TAPS):
            if caps[o] > 0 or o == _TAPS[-1]:
                caps[o] += pad
                break
        gnc += pad
        offs = {}
        cur = 0
        for o in _TAPS:
            offs[o] = cur
            cur += caps[o]
        assert cur == gnc
        # per-core index streams (non-center only)
        gsrc = np.full((NCORES, gnc), N, dtype=np.int64)    # pad -> zero col
        sdst = np.full((NCORES, gnc), ROWS, dtype=np.int64)  # pad -> dump row
        for cc in range(NCORES):
            g = gidx[rowmap[cc]]
            for o in _TAPS:
                if caps[o] == 0:
                    continue
                v = np.nonzero(g[:, o] >= 0)[0]
                gsrc[cc, offs[o]:offs[o] + len(v)] = g[v, o]
                sdst[cc, offs[o]:offs[o] + len(v)] = v
        return caps, gnc, offs, gsrc, sdst

    # the ucode scatter-add collapses duplicate dsts closer than 32 slots in
    # one call: widen the earlier tap's span until no real-real pair violates
    for _ in range(32):
        caps2, gnc, offs, gsrc, sdst = _layout(caps)
        bad_tap = None
        for cc in range(NCORES):
            d, real = sdst[cc], sdst[cc] < ROWS
            for w in range(1, 32):
                m = np.nonzero((d[:-w] == d[w:]) & real[:-w] & real[w:])[0]
                if len(m):
                    p = int(m[0])
                    for o in _TAPS:
                        if caps2[o] and offs[o] <= p < offs[o] + caps2[o]:
                            bad_tap = o
                            break
                    break
            if bad_tap is not None:
                break
        if bad_tap is None:
            break
        caps[bad_tap] += 32
    else:
        raise RuntimeError("could not satisfy scatter-add min-distance")
    caps = caps2

    # ucode scatter chunks (multiples of 16): per-call cost is
    # max(accumulator_free, 2*chunk) * 1.39ns + 95ns. uchunks_spec gives the
    # boundaries of all but the last chunk.
    uchunks = []
    p = 0
    for b in uchunks_spec:
        b = min(b, gnc)
        if b > p:
            uchunks.append((p, b))
            p = b
    if p < gnc:
        uchunks.append((p, gnc))

    # The inputs ride in a few packed DMA buffers (every HWDGE DMA costs
    # ~630ns on the shared HWDGE engine plus a 900ns completion-semaphore
    # propagation, and all transfers serialize on the shared DMA engines —
    # so few, purpose-ordered DMAs win). Buffer i carries the weights of the
    # taps whose span STARTS in its edge-column range (so every matmul
    # segment's weights arrive no later than its edge columns), followed by
    # those edge columns; the last buffer also carries the center-tap
    # weights and the transposed self features.
    cuts = [c for c in icuts if c < gnc] + [gnc]
    bufspec = []
    p = 0
    for bi, c1 in enumerate(cuts):
        taps = [o for o in _TAPS if caps[o] and p <= offs[o] < c1]
        bufspec.append(dict(c0=p, c1=c1, taps=taps, has_self=False))
        p = c1
    bufspec[-1]["taps"].append(13)
    bufspec[-1]["has_self"] = True
    gchunks = [(b["c0"], b["c1"]) for b in bufspec]

    # matmul segments per psum_block of the edge stream: (block, c0, c1, tap)
    # (c0/c1 are block-local columns; products go to PSUM columns, so no
    # partition-alignment constraints).
    def _clip_spans(spans, total, extra_cuts=()):
        cuts = sorted(set(range(0, total + psum_block, psum_block))
                      | set(extra_cuts))
        nblk = -(-total // psum_block)
        out = [[] for _ in range(nblk)]
        for a, bnd, o in spans:
            p = a
            while p < bnd:
                lim = min(bnd, min(c for c in cuts if c > p))
                blk = p // psum_block
                out[blk].append((p - blk * psum_block, lim - blk * psum_block, o))
                p = lim
        return out

    segs_nc = _clip_spans(
        [(offs[o], offs[o] + caps[o], o) for o in _TAPS if caps[o] > 0], gnc,
        extra_cuts=[c0 for (c0, _c1) in gchunks])
    segs_c = _clip_spans([(0, ROWS, 13)], ROWS)

    ixs = np.concatenate([_wrap16(sdst[cc])[None] for cc in range(NCORES)],
                         axis=0)  # [NCORES, 128, gnc//16]

    # tap -> (buffer id, weight column offset within buffer); buffer widths
    wcol = {}
    for bi, b in enumerate(bufspec):
        for i, o in enumerate(b["taps"]):
            wcol[o] = (bi, i * C)
        b["nw"] = len(b["taps"]) * C
        b["W"] = b["nw"] + (b["c1"] - b["c0"]) + (ROWS if b["has_self"] else 0)

    return dict(gnc=gnc, segs_nc=segs_nc, segs_c=segs_c, gchunks=gchunks,
                uchunks=uchunks, psum_block=psum_block, ixs=ixs, gsrc=gsrc,
                rowmap=rowmap,
                bufspec=bufspec, wcol=wcol, queues=queues, warm_pre=warm_pre,
                bridges=dict(bridges), copy_split=copy_split, copy_eng=copy_eng,
                sig=(gnc, tuple(caps[o] for o in _TAPS), tuple(uchunks_spec),
                     psum_block, tuple(cuts), queues, warm_pre,
                     tuple(sorted(dict(bridges).items())), copy_split, copy_eng))


# ---------------- Bass device program ----------------
_CACHED = {}


def _elide_same_engine_waits(nc):
    """Drop semaphore waits that are implied by in-order engine execution:
    a wait whose semaphore is incremented ONLY by earlier non-DMA
    instructions on the SAME engine is always satisfied by the time the
    instruction executes (engines run their queues in order; only DMA
    completions are asynchronous to the issuing engine). The tile
    framework's per-engine clock sems produce such waits, e.g. the second
    scatter chunk waiting on the first one's Pool clock tick."""
    import concourse.mybir as mybir

    for fn in nc.m.functions:
        for bb in fn.blocks:
            insts = list(bb.instructions)
            sem_updaters = {}  # sem id -> [(engine, engine_pos, inc|None, is_async)]
            engpos = {}
            counter = {}
            for inst in insts:
                e = inst.engine
                p = counter.get(e, 0)
                counter[e] = p + 1
                engpos[inst.name] = p
                si = inst.sync_info
                if si is None:
                    continue
                is_async = "DMA" in type(inst).__name__ or "Dma" in type(inst).__name__
                for u in (si.on_update or []):
                    if u.update_mode == "sem-inc":
                        inc = 1
                    elif u.update_mode == "sem-add-imm":
                        inc = u.update_value
                    else:
                        inc = None  # barrier-style (sub) updates: never elide
                    sem_updaters.setdefault(u.id, []).append((e, p, inc, is_async))
            for inst in insts:
                si = inst.sync_info
                if si is None or not si.on_wait:
                    continue
                e, p = inst.engine, engpos[inst.name]
                keep = []
                for w in si.on_wait:
                    ups = sem_updaters.get(w.id, [])
                    drop = False
                    if w.wait_mode == "sem-ge-imm" and ups:
                        ok = all(ue == e and ui is not None and not ua
                                 for (ue, _up, ui, ua) in ups)
                        if ok:
                            tot = sum(ui for (_ue, up, ui, _ua) in ups if up < p)
                            drop = tot >= w.wait_value
                    if not drop:
                        keep.append(w)
                if len(keep) != len(list(si.on_wait)):
                    inst.sync_info = mybir.SyncInfo(
                        on_wait=keep, on_update=list(si.on_update or []))


def _split_multiwait(nc):
    """Walrus encodes at most one sync wait per instruction. Hoist extra
    waits onto same-engine NOPs inserted just before."""
    import concourse.mybir as mybir

    ctr = 0
    for fn in nc.m.functions:
        for bb in fn.blocks:
            insts = bb.instructions
            orig = list(insts)
            newlist = []
            for inst in orig:
                si = inst.sync_info
                waits = list(si.on_wait or []) if si is not None else []
                if len(waits) >= 2:
                    for w in waits:
                        nop = mybir.InstNoOp(name=f"I-wsplit{ctr}", ins=[], outs=[])
                        ctr += 1
                        nop.engine = inst.engine
                        nop.sync_info = mybir.SyncInfo(on_wait=[w], on_update=[])
                        # register so CoreSim's race detector sees it (its
                        # fake-sem-update pass walks inst_map, not the blocks)
                        nc.inst_map[nop.name] = nop
                        newlist.append(nop)
                    inst.sync_info = mybir.SyncInfo(
                        on_wait=[], on_update=list(si.on_update or []))
                newlist.append(inst)
            insts.clear()
            insts.extend(newlist)


def _build_conv_program(plan):
    import concourse.bass as bass
    import concourse.mybir as mybir
    import concourse.tile as tile
    from concourse import library_config

    nc = bass.Bass("TRN2")
    f32 = mybir.dt.float32
    bf16 = mybir.dt.bfloat16
    i16 = mybir.dt.int16

    gnc = plan["gnc"]
    segs_nc = plan["segs_nc"]
    segs_c = plan["segs_c"]
    uchunks = plan["uchunks"]
    pb = plan["psum_block"]
    wcol = plan["wcol"]
    bufspec = plan["bufspec"]
    queues = plan["queues"]
    warm_pre = plan["warm_pre"]
    bridges = plan["bridges"]
    copy_split = plan["copy_split"]
    copy_eng = plan["copy_eng"]
    ixw = gnc // 16

    bufds = [nc.dram_tensor(f"buf{bi}", [C, b["W"]], bf16, kind="ExternalInput")
             for bi, b in enumerate(bufspec)]
    ixs = nc.dram_tensor("ixs", [128, ixw], i16, kind="ExternalInput")
    Y = nc.dram_tensor("Y", [C, ROWS * 2], bf16, kind="ExternalOutput")
    CEN = nc.dram_tensor("CEN", [C, ROWS], bf16, kind="ExternalOutput")

    from contextlib import ExitStack
    with ExitStack() as ctx:
        tc = ctx.enter_context(
            tile.TileContext(nc, linearize=os.environ.get("KERNEL_LINEARIZE", "0") == "1"))
        const = ctx.enter_context(tc.tile_pool(name="const", bufs=1))
        psum_pool = ctx.enter_context(tc.tile_pool(name="pp", bufs=4, space="PSUM"))
        dmy_pool = ctx.enter_context(tc.tile_pool(name="dmy", bufs=1, space="PSUM"))
        scat_pool = ctx.enter_context(tc.tile_pool(name="scat", bufs=1))

        nc.gpsimd.load_library(library_config.mlp)

        # DMAs in. Only SP and Activation have HWDGE queues; gpsimd can also
        # start (SWDGE) DMAs and is idle until the first scatter. The packed
        # buffers ride the queues given by plan["queues"]; the scatter idxs
        # ride the gpsimd SWDGE queue (they only gate the ucode chain).
        # PE p-state warm-up (optional; off by default — the cost model runs
        # matmuls at 1.2 GHz until 3us of continuous PE busy, but dummy-chain
        # warming measured net-neutral to negative here since the matmuls are
        # off the critical path).
        if warm_pre or bridges:
            dmy = const.tile([C, 64], bf16)
            nc.scalar.memzero(dmy[:])
            pd = dmy_pool.tile([64, 64], f32)

        def emit_warm(n):
            for _ in range(n):
                nc.tensor.matmul(pd[0:64, :], lhsT=dmy[0:C, 0:64],
                                 rhs=dmy[0:C, 0:64],
                                 start=True, stop=True, skip_group_check=True)

        # accumulator [C, ROWS+8, 2] bf16 (j=0 real, j=1 dead lane for d=2,
        # rows >= ROWS take the pad-slot adds), zeroed up front so the
        # scatter chain is independent of the center/self path. The j=1
        # lanes and dump rows accumulate garbage; the host only reads j=0 of
        # rows < ROWS. scat j=1 lanes are never written (uninitialized),
        # matching the accumulator's dead lanes.
        ysb = scat_pool.tile([C, ROWS + 8, 2], bf16)
        nc.vector.memset(ysb[:], 0.0)
        scat = scat_pool.tile([C, gnc, 2], bf16)
        cen = scat_pool.tile([C, ROWS], bf16)

        ixb = const.tile([128, ixw], i16)
        nc.gpsimd.dma_start(ixb[:], ixs[:])
        qmap = {"S": nc.sync, "A": nc.scalar}
        bufs = []
        for bi, b in enumerate(bufspec):
            t = const.tile([C, b["W"]], bf16, tag=f"buf{bi}")
            qmap[queues[bi]].dma_start(t[:], bufds[bi][:])
            bufs.append(t)

        emit_warm(warm_pre)

        def wslice(o):
            bi, off = wcol[o]
            return bufs[bi][0:C, off:off + C]

        def rhs_slice(col, ln):
            for bi, b in enumerate(bufspec):
                if b["c0"] <= col < b["c1"]:
                    off = b["nw"] + col - b["c0"]
                    return bufs[bi][0:C, off:off + ln]
            raise AssertionError(col)

        def self_slice(col, ln):
            b = bufspec[-1]
            off = b["nw"] + (b["c1"] - b["c0"])
            return bufs[-1][0:C, off + col:off + col + ln]

        # products: out[oc, edge] = sum_ic W[ic, o, oc] * feat[ic, edge]
        # (weights stationary, feature columns moving -> arbitrary column
        # spans, no PSUM partition-alignment constraints)
        ncopies = 0

        def emit_copy(eng, dst, src):
            if eng is nc.scalar:
                eng.activation(dst, src, mybir.ActivationFunctionType.Copy)
            else:
                eng.tensor_copy(dst, src)

        def emit_block(blk, seglist, lhs_fn, out_tile):
            nonlocal ncopies
            lo = blk * pb
            width = max(c1 for (c0, c1, _o) in seglist)
            pp = psum_pool.tile([C, pb], f32, tag="pp")
            for (c0, c1, o) in seglist:
                nc.tensor.matmul(
                    pp[0:C, c0:c1],
                    lhsT=wslice(o),
                    rhs=lhs_fn(lo + c0, c1 - c0),
                    start=True, stop=True, skip_group_check=True)
            # copy psum -> bf16, split into pieces across copy engines so
            # the scatter's gate closes sooner. In "dvp" mode the blocks of
            # the FIRST scatter chunk split DVE+gpsimd: the gpsimd halves
            # precede the first scatter in its own engine order (no extra
            # semaphore hop), and gpsimd is idle until then anyway. Later
            # blocks stay off gpsimd so they don't wedge between scatters.
            dvp = (copy_eng == "dvp" and out_tile is scat
                   and lo + width <= uchunks[0][1])
            pieces = 2 if dvp else max(1, min(copy_split, width // 128))
            step = -(-width // pieces)
            for pi in range(pieces):
                a, bnd = pi * step, min((pi + 1) * step, width)
                if dvp:
                    eng = nc.vector if pi == 0 else nc.gpsimd
                elif copy_eng in ("dve", "dvp"):
                    eng = nc.vector
                elif copy_eng == "act":
                    eng = nc.scalar
                else:
                    eng = nc.vector if (ncopies % 2 == 0) else nc.scalar
                if out_tile is cen:
                    dst = out_tile[:, lo + a:lo + bnd]
                else:
                    dst = out_tile[:, lo + a:lo + bnd, 0]
                emit_copy(eng, dst, pp[0:C, a:bnd])
                ncopies += 1

        def ucode_of(col):
            for ui, (u0, u1) in enumerate(uchunks):
                if u0 <= col < u1:
                    return ui
            raise AssertionError(col)

        # non-center blocks fill the ucode add stream; alternate copy
        # engines within each ucode chunk
        emitted_isa = set()
        for blk, seglist in enumerate(segs_nc):
            emit_block(blk, seglist, rhs_slice, scat)
            # keep the PE hot across data-arrival gaps
            emit_warm(bridges.get(blk, 0))
            # emit each scatter chunk as soon as its blocks are done so the
            # Pool SEQ order matches data availability
            nxt = (blk + 1) * pb
            for ui, (u0, u1) in enumerate(uchunks):
                if ui not in emitted_isa and nxt >= u1:
                    nc.gpsimd.scatter_add(
                        ysb[:], ixb[0:C, u0 // 16:u1 // 16],
                        scat[:, u0:u1, :], C, ROWS + 8, 2, u1 - u0)
                    emitted_isa.add(ui)
        assert emitted_isa == set(range(len(uchunks)))

        # center blocks go to their own tile (host adds it to the scatter
        # accumulator), keeping the self path off the ucode chain
        for blk, seglist in enumerate(segs_c):
            emit_block(blk, seglist, self_slice, cen)

        nc.scalar.dma_start(CEN[:], cen[:])
        nc.sync.dma_start(Y[:], ysb[:, 0:ROWS, :])
    if os.environ.get("KERNEL_ELIDE_WAITS", "1") == "1":
        _elide_same_engine_waits(nc)
    if os.environ.get("KERNEL_SPLIT_MULTIWAIT", "1") == "1":
        _split_multiwait(nc)
    # Raw Bass skips Bacc's codegen_inst_isa_subclasses pass; without it the
    # NEFF compiler sees empty .instr bytes for extended-ISA instructions
    # (e.g. the library reload) and fails with "ISA wrong length".
    mybir.codegen_inst_isa_subclasses(nc)
    return nc


def _run_conv(feats_f32, plan, w_flat):
    """feats_f32 [N, C] f32, w_flat [27, C, C] f32 -> conv output [N, C] f32
    (no bias; SubMConv3d has none)."""
    from concourse.bass_utils import run_bass_kernel_spmd

    if _CACHED.get("sig") != plan["sig"]:
        _CACHED["nc"] = _build_conv_program(plan)
        _CACHED["sig"] = plan["sig"]
    nc = _CACHED["nc"]

    import ml_dtypes
    fpb = np.ascontiguousarray(feats_f32).astype(ml_dtypes.bfloat16)
    wb = np.ascontiguousarray(np.transpose(w_flat, (1, 0, 2)))  # [ic, o, oc]
    wbb = wb.astype(ml_dtypes.bfloat16)

    gnc = plan["gnc"]
    gsrc = plan["gsrc"]
    in_maps = []
    for cc in range(NCORES):
        edgeT = np.zeros((C, gnc), dtype=ml_dtypes.bfloat16)
        real = gsrc[cc] < N
        edgeT[:, real] = fpb[gsrc[cc][real]].T
        m = {"ixs": plan["ixs"][cc]}
        for bi, b in enumerate(plan["bufspec"]):
            parts = [wbb[:, o, :] for o in b["taps"]]
            parts.append(edgeT[:, b["c0"]:b["c1"]])
            if b["has_self"]:
                parts.append(fpb[plan["rowmap"][cc], :].T)
            buf = np.concatenate(parts, axis=1)
            assert buf.shape[1] == b["W"]
            m[f"buf{bi}"] = np.ascontiguousarray(buf)
        in_maps.append(m)
    trace = os.environ.get("KERNEL_TRACE", "") == "1"
    res = run_bass_kernel_spmd(nc, in_maps, core_ids=list(range(NCORES)), trace=trace)
    if trace and res.exec_time_ns is not None:
        print(f"HW exec time: {res.exec_time_ns} ns")
        _CACHED.setdefault("exec_ns", []).append(res.exec_time_ns)
    out = np.empty((N, C), dtype=np.float32)
    for cc in range(NCORES):
        Yc = np.asarray(res.results[cc]["Y"]).astype(np.float32).reshape(C, ROWS, 2)
        Cc = np.asarray(res.results[cc]["CEN"]).astype(np.float32)
        out[plan["rowmap"][cc]] = (Yc[:, :, 0] + Cc).T
    return out


def _conv_host(feats_f32, plan, w_flat):
    """Host fallback/validation path for the conv (numpy, fp32)."""
    del plan
    gidx = _build_gather(_CACHED["indices"])
    acc = np.zeros((N, C), dtype=np.float32)
    for o in range(27):
        v = gidx[:, o] >= 0
        acc[v] += feats_f32[gidx[v, o]] @ w_flat[o]
    return acc


def kernel(**inputs):
    inputs = {k: np.asarray(v) for k, v in inputs.items()}
    fused = _host_pre(
        inputs['x'], inputs['indices'], inputs['fp_w'], inputs['fp_b'], inputs['fp_g'],
        inputs['fp_be'], inputs['att_w1'], inputs['att_b1'], inputs['att_w2'], inputs['att_b2'],
        inputs['ff_w1'], inputs['ff_b1'], inputs['ff_g'], inputs['ff_be'], inputs['ff_w2'],
        inputs['ff_b2'], inputs['sa_w1'], inputs['sa_b1'], inputs['sa_w2'], inputs['sa_b2'],
        inputs['fj_w1'], inputs['fj_b1'], inputs['fj_g'], inputs['fj_be'], inputs['fj_w2'],
        inputs['fj_b2'], inputs['proj_w'], inputs['proj_g'], inputs['proj_be'], inputs['lw_w'],
        inputs['lw_g'], inputs['lw_be'], inputs['w_w'], inputs['adp_w'], inputs['fuse_w'],
        inputs['fuse_g'], inputs['fuse_be'])

    _CACHED["indices"] = inputs['indices']
    key = inputs['indices'].tobytes()
    if _CACHED.get("plan_key") != key:
        _CACHED["plan"] = _build_edge_plan(inputs['indices'])
        _CACHED["plan_key"] = key
    plan = _CACHED["plan"]

    w1 = inputs['conv1_w'].reshape(27, C, C).astype(np.float32)
    w2 = inputs['conv2_w'].reshape(27, C, C).astype(np.float32)

    conv = _conv_host if os.environ.get("KERNEL_HOST_CONV", "") == "1" else _run_conv

    raw1 = conv(fused, plan, w1)
    f1 = _relu(_bn(raw1, inputs['bn1_g'], inputs['bn1_be']))
    raw2 = conv(f1, plan, w2)
    f2 = _bn(raw2, inputs['bn2_g'], inputs['bn2_be'])
    return _relu(f2 + fused).astype(np.float32)


# revision 43
# speedup vs baseline: 1.0108x; 1.0062x over previous
"""Trainium2 kernel for nn_BasicBlock_53171695125036 (gnn_message_passing).

Split of work:
  - The two SubMConv3d sparse convolutions (the dominant FLOPs) run on all 8
    NeuronCores as edge-list GEMM + on-device scatter-add Bass kernels.
    The edge plan (which neighbor feeds which output row through which tap)
    is compile-time data derived from `indices`, so the HOST packs the
    gathered neighbor feature columns [C, gnc] per core (input marshalling,
    like the transposed self-feature tile) and the device loads them with
    plain HWDGE DMAs — this removes the per-launch SWDGE gather whose
    994 ns/instruction descriptor generation serialized on the gpsimd
    engine ahead of the scatter ucode in the previous design.
  - Device per launch: 3 packed input DMAs (each HWDGE DMA costs ~630 ns on
    the shared HWDGE engine + ~900 ns completion-sem propagation, and all
    transfers serialize on the shared DMA engines, so few purpose-ordered
    buffers win) -> per-tap matmuls with weights stationary (lhsT)
    producing [out_ch, edge] products in PSUM -> psum-block copies to a
    bf16 stream -> gpsimd InstScatterAdd ucode accumulates the stream into
    a zeroed accumulator [C, ROWS+8, 2] in two chunks -> DMA out.  The
    center tap (every point itself) goes through its own psum->sbuf tile
    and is summed with the scatter accumulator on host, keeping the self
    products (and their DMA) off the scatter chain.
  - Rows are assigned to cores by a greedy balancer that equalizes per-tap
    edge counts across cores (the SPMD layout pads every tap span to the
    worst core, so balancing shrinks the padded stream to its floor —
    gnc 2016 -> 1760 here); the host un-permutes the output for free.
  - The irregular per-point pipeline (CMPFE MLPs, integer kNN selection,
    voxel clustering, segment softmax aggregation) is computed on host in
    fp32, bit-faithful to the jax reference where it is discretely
    sensitive (cluster ids, kNN sets).
  - BatchNorm between the two convs needs global batch stats, so the convs
    are two launches of ONE compiled program with host stat combination
    in between. (Fusing both convs into one launch would need on-device
    global BN stats = cross-core exchange; collectives cost a flat 15 us
    in the cost model and remote_dma is unmodeled in no_exec TimelineSim,
    so the two-launch structure stands.)

Hardware facts established by experiment (axon-tunneled trn2):
  - dma_scatter_add (DMA engines) loses concurrent read-modify-writes when
    one instruction carries duplicate destination rows; adds ACROSS
    serialized instructions are exact. Unusable for this conv (every dst
    row receives ~3.3 tap contributions).
  - the InstScatterAdd gpsimd ucode processes indices in 32-wide vector
    batches: duplicate dsts >= 32 slots apart accumulate exactly, closer
    ones collapse. The edge plan guarantees the spacing (unique ascending
    dsts within a tap, >= 32-slot tap spans, pad slots aimed at a dump
    row); a checker widens spans if any input ever violates it. CoreSim's
    Python interp of this op uses buffered fancy-index += (duplicates
    collapse), so CoreSim under-reports accuracy here — device is truth.
  - raw Bass needs mybir.codegen_inst_isa_subclasses() before the NEFF
    compile and one sync-wait per instruction (_split_multiwait), with the
    hoisted-wait NOPs registered in nc.inst_map for the race detector.
"""

import os
import sys

import numpy as np

for _p in ("/opt/trn_rl_repo",):
    if _p not in sys.path and os.path.isdir(_p):
        sys.path.insert(0, _p)

N = 6144
C = 96
B = 2
D = H = W = 32
K = 16
DEPTH = 4
NCORES = 8
ROWS = N // NCORES  # 768
GRID_OPTS = np.array([[0.1, 0.1, 0.1], [0.4, 0.4, 0.4], [0.2, 0.2, 0.2]], dtype=np.float32)
BN_EPS = 1e-5

F32 = np.float32


def _bn(x, g, b):
    m = x.mean(0)
    v = x.var(0)
    return (x - m) * (1.0 / np.sqrt(v + F32(BN_EPS))) * g + b


def _relu(x):
    return np.maximum(x, F32(0.0))


def _sigmoid(x):
    return F32(1.0) / (F32(1.0) + np.exp(-x))


def _softmax(x, axis):
    e = np.exp(x - x.max(axis=axis, keepdims=True))
    return e / e.sum(axis=axis, keepdims=True)


def _seg_sum(x, seg):
    out = np.zeros((N, x.shape[1]), dtype=x.dtype)
    np.add.at(out, seg, x)
    return out


def _knn_idx(coord_i, batch):
    """Exact mirror of the reference top-k: all d2 values are small ints,
    exact in fp32, so selection == ascending (d2, index) lexicographic."""
    sq = (coord_i * coord_i).sum(1)  # int64
    d2 = sq[:, None] + sq[None, :] - 2 * (coord_i @ coord_i.T)
    same = batch[None, :] == batch[:, None]
    np.fill_diagonal(same, False)
    BIG = np.int64(1 << 40)
    key = d2 * 8192 + np.arange(N, dtype=np.int64)[None, :]
    key = np.where(same, key, BIG)
    part = np.argpartition(key, K, axis=1)[:, :K]
    pk = np.take_along_axis(key, part, axis=1)
    srt = np.argsort(pk, axis=1)
    return np.take_along_axis(part, srt, axis=1)  # [N, K]


def _host_pre(x, indices, fp_w, fp_b, fp_g, fp_be, att_w1, att_b1, att_w2, att_b2,
              ff_w1, ff_b1, ff_g, ff_be, ff_w2, ff_b2, sa_w1, sa_b1, sa_w2, sa_b2,
              fj_w1, fj_b1, fj_g, fj_be, fj_w2, fj_b2,
              proj_w, proj_g, proj_be, lw_w, lw_g, lw_be, w_w, adp_w,
              fuse_w, fuse_g, fuse_be):
    # ---- CMPFE ----
    p = _relu(_bn(x @ fp_w.T + fp_b, fp_g, fp_be))
    cd, cl, nm = p[:, :3], p[:, 3:6], p[:, 6:9]

    def _att(f, i):
        h = _relu(f @ att_w1[i].T + att_b1[i])
        return _sigmoid(h @ att_w2[i].T + att_b2[i])

    enh = np.concatenate([cd, cl * _att(cl, 0), nm * _att(nm, 1)], axis=1)
    fu = _relu(_bn(enh @ ff_w1.T + ff_b1, ff_g, ff_be)) @ ff_w2.T + ff_b2
    sem = _sigmoid(_relu(fu @ sa_w1.T + sa_b1) @ sa_w2.T + sa_b2)
    feat = fu * sem + x * (F32(1.0) - sem)

    # ---- PFAS geometry ----
    coord_i = indices[:, 1:].astype(np.int64)
    coord = indices[:, 1:].astype(F32)
    batch = indices[:, 0]
    idx = _knn_idx(coord_i, batch)
    nbr = coord[idx]  # [N, K, 3]
    cent = nbr - nbr.mean(axis=1, keepdims=True)
    cov = np.einsum('nkd,nke->nde', cent, cent) / F32(K - 1)
    S = np.linalg.svd(cov, compute_uv=False)
    Sn = S / (S.sum(axis=1, keepdims=True) + F32(1e-6))
    linearity = Sn[:, 0:1] - (Sn[:, 1] + Sn[:, 2])[:, None]
    diff = coord[:, None, :] - nbr  # [N,K,3]
    d2f = (diff * diff).sum(-1)
    nd = np.sqrt(np.maximum(d2f, F32(1e-12)))
    mean_dist = nd.mean(axis=1, keepdims=True)
    density = F32(1.0) / (mean_dist + F32(1e-6))
    fl = _relu(_bn(feat @ fj_w1.T + fj_b1, fj_g, fj_be)) @ fj_w2.T + fj_b2
    fp_ = _softmax(fl, axis=1)
    tower = (density * 2.0 + fp_[:, 0:1]) / 3.0
    backg = (np.maximum(F32(1.0) - linearity, F32(1.0) - density) + fp_[:, 1:2]) / 3.0
    line = (linearity * 2.0 + fp_[:, 2:3]) / 3.0
    lg = GRID_OPTS[2] * np.array([1.0, 1.0, 5.0], F32)
    grid_sizes = (tower * GRID_OPTS[0] + backg * GRID_OPTS[1] + line * lg + F32(1e-6)).astype(F32)

    gm = grid_sizes.mean(axis=1)
    order = np.argsort(gm, kind='stable')
    reps = [grid_sizes[order[100:200]].mean(0),
            grid_sizes[order[::-1][:100]].mean(0),
            grid_sizes[order[:100]].mean(0)]

    start = coord.min(axis=0)

    def _cluster(size):
        size = np.clip(size, F32(1e-6), None).astype(F32)
        c = np.clip(np.floor((coord - start) / size).astype(np.int64), 0, 4095)
        mx = c.max(axis=0) + 1
        ids = ((batch.astype(np.int64) * mx[0] + c[:, 0]) * mx[1] + c[:, 1]) * mx[2] + c[:, 2]
        _, inv = np.unique(ids, return_inverse=True)
        return inv.reshape(-1)

    branch_feats = []
    for i in range(DEPTH - 1):
        seg = _cluster(reps[i])
        cnt = np.maximum(_seg_sum(np.ones((N, 1), feat.dtype), seg), F32(1.0))
        pw = _relu(_bn(feat @ lw_w[i].T, lw_g[i], lw_be[i]))
        pw = pw - (_seg_sum(pw, seg) / cnt)[seg]
        pw = pw @ w_w[i].T
        pw = np.exp(pw - pw.max())
        pw = pw / (_seg_sum(pw, seg)[seg] + F32(1e-6))
        pf = _relu(_bn(feat @ proj_w[i].T, proj_g[i], proj_be[i])) * pw
        branch_feats.append(_seg_sum(pf, seg)[seg])
    adp = _softmax(feat @ adp_w.T, axis=1)
    agg = np.einsum('nc,ncd->nd', adp, np.stack(branch_feats, 1))
    last = _relu(_bn(feat @ proj_w[-1].T, proj_g[-1], proj_be[-1]))
    fused = _relu(_bn(np.concatenate([last, agg], 1) @ fuse_w.T, fuse_g, fuse_be)) + feat
    return fused.astype(F32)


def _build_gather(indices):
    """[N, 27] int64 gather map for 3x3x3 SAME conv; -1 == inactive site."""
    lut = -np.ones((B, D + 2, H + 2, W + 2), dtype=np.int64)
    bi, zi, yi, xi = indices[:, 0], indices[:, 1], indices[:, 2], indices[:, 3]
    lut[bi, zi + 1, yi + 1, xi + 1] = np.arange(N)
    gidx = np.empty((N, 27), dtype=np.int64)
    o = 0
    for dz in range(3):
        for dy in range(3):
            for dx in range(3):
                gidx[:, o] = lut[bi, zi + dz, yi + dy, xi + dx]
                o += 1
    return gidx


# ---------------- edge plan (SPMD-uniform sparse layout) ----------------
#
# The center tap (o=13) is every point itself: its features load as one
# contiguous transposed slice and its products go to a separate [C, ROWS]
# tile (identity dst order) summed with the scatter accumulator on host.
#
# The other 26 taps form a "non-center" edge stream of length gnc (multiple
# of 16): tap o occupies a fixed column span of cap[o] (max real edge count
# across cores, so the layout is SPMD-uniform). Pad slots have zero feature
# columns (host packs zeros) and dst = dump row: their products are exactly
# zero, so scatter-adding them is a no-op.
#
# Accumulation runs through the gpsimd InstScatterAdd ucode. Two measured
# hardware facts shape this:
#   * the DMA scatter-add engine loses concurrent read-modify-writes to the
#     same row (any duplicate dst in one instruction), so it is unusable for
#     this conv;
#   * the ucode scatter-add processes indices in 32-wide vector batches:
#     duplicate dsts >= 32 positions apart accumulate exactly, closer ones
#     collapse. Within a tap dsts are unique and ascending, and same-dst
#     edges of different taps sit ~cap (>= 32) positions apart; pad slots
#     point at a dummy accumulator row so they cannot collide with real
#     edges. _build_edge_plan verifies and widens caps if needed.
# The ucode layout needs an even inner dim d: the accumulator is
# [C, ROWS+8, 2] with the real value at j=0, a dead j=1 lane, and rows
# >= ROWS as the pad dump. It starts from a memset-0 tile, so the scatter
# chain never waits on the self-feature path.

_TAPS = [o for o in range(27) if o != 13]


def _balance_rows(gidx):
    """Greedy row->core assignment (equal 768-row shards) minimizing
    sum_o max_cc count(cc, o) — the padded edge-stream length is set by the
    worst core per tap, so balancing tap counts across cores shrinks gnc.
    The host un-permutes the output for free."""
    present = gidx[:, _TAPS] >= 0  # [N, 26]
    order = np.argsort(-present.sum(1), kind="stable")
    counts = np.zeros((NCORES, len(_TAPS)), dtype=np.int64)
    fill = np.zeros(NCORES, dtype=np.int64)
    rowmap = np.empty((NCORES, ROWS), dtype=np.int64)
    for r in order:
        s = present[r]
        cur_max = counts.max(axis=0)
        best, best_cost = None, None
        for cc in range(NCORES):
            if fill[cc] >= ROWS:
                continue
            # increase in sum-of-maxes if row r goes to core cc
            cost = (np.maximum(counts[cc] + s, cur_max).sum(), counts[cc][s].sum(), fill[cc])
            if best is None or cost < best_cost:
                best, best_cost = cc, cost
        counts[best] += s
        rowmap[best, fill[best]] = r
        fill[best] += 1
    assert (fill == ROWS).all()
    return rowmap


def _wrap16(a):
    """[L] -> [128, L//16] wrapped index layout (idx i at partition
    i%16, col i//16, replicated to the 8 gpsimd cores)."""
    w = a.reshape(-1, 16).T  # [16, L//16]
    return np.ascontiguousarray(np.tile(w, (8, 1)).astype(np.int16))


def _build_edge_plan(indices, uchunks_spec=(784,), psum_block=392,
                     icuts=(512, 880), queues="SAA", warm_pre=0,
                     bridges=(), copy_split=1, copy_eng="alt", balance=True):
    gidx = _build_gather(indices)  # [N, 27], -1 invalid
    if balance:
        rowmap = _balance_rows(gidx)
    else:
        rowmap = np.arange(N, dtype=np.int64).reshape(NCORES, ROWS)
    counts = np.zeros(27, dtype=np.int64)
    for o in _TAPS:
        v = gidx[:, o] >= 0
        counts[o] = max(v[rowmap[c]].sum() for c in range(NCORES))
    # caps need no alignment (matmul spans and idx values are arbitrary;
    # only chunk boundaries are 16-aligned) — but >= 32 when non-empty so
    # same-dst edges of neighboring taps stay >= 32 apart for the ucode
    caps = {o: int(max(counts[o], 32)) if counts[o] else 0 for o in _TAPS}

    def _layout(caps):
        gnc = sum(caps.values())
        caps = dict(caps)
        pad = (-gnc) % 16  # idx wrap needs a multiple of 16
        for o in reversed(_TAPS):
            if caps[o] > 0 or o == _TAPS[-1]:
                caps[o] += pad
                break
        gnc += pad
        offs = {}
        cur = 0
        for o in _TAPS:
            offs[o] = cur
            cur += caps[o]
        assert cur == gnc
        # per-core index streams (non-center only)
        gsrc = np.full((NCORES, gnc), N, dtype=np.int64)    # pad -> zero col
        sdst = np.full((NCORES, gnc), ROWS, dtype=np.int64)  # pad -> dump row
        for cc in range(NCORES):
            g = gidx[rowmap[cc]]
            for o in _TAPS:
                if caps[o] == 0:
                    continue
                v = np.nonzero(g[:, o] >= 0)[0]
                gsrc[cc, offs[o]:offs[o] + len(v)] = g[v, o]
                sdst[cc, offs[o]:offs[o] + len(v)] = v
        return caps, gnc, offs, gsrc, sdst

    # the ucode scatter-add collapses duplicate dsts closer than 32 slots in
    # one call: widen the earlier tap's span until no real-real pair violates
    for _ in range(32):
        caps2, gnc, offs, gsrc, sdst = _layout(caps)
        bad_tap = None
        for cc in range(NCORES):
            d, real = sdst[cc], sdst[cc] < ROWS
            for w in range(1, 32):
                m = np.nonzero((d[:-w] == d[w:]) & real[:-w] & real[w:])[0]
                if len(m):
                    p = int(m[0])
                    for o in _TAPS:
                        if caps2[o] and offs[o] <= p < offs[o] + caps2[o]:
                            bad_tap = o
                            break
                    break
            if bad_tap is not None:
                break
        if bad_tap is None:
            break
        caps[bad_tap] += 32
    else:
        raise RuntimeError("could not satisfy scatter-add min-distance")
    caps = caps2

    # ucode scatter chunks (multiples of 16): per-call cost is
    # max(accumulator_free, 2*chunk) * 1.39ns + 95ns. uchunks_spec gives the
    # boundaries of all but the last chunk.
    uchunks = []
    p = 0
    for b in uchunks_spec:
        b = min(b, gnc)
        if b > p:
            uchunks.append((p, b))
            p = b
    if p < gnc:
        uchunks.append((p, gnc))

    # The inputs ride in a few packed DMA buffers (every HWDGE DMA costs
    # ~630ns on the shared HWDGE engine plus a 900ns completion-semaphore
    # propagation, and all transfers serialize on the shared DMA engines —
    # so few, purpose-ordered DMAs win). Buffer i carries the weights of the
    # taps whose span STARTS in its edge-column range (so every matmul
    # segment's weights arrive no later than its edge columns), followed by
    # those edge columns; the last buffer also carries the center-tap
    # weights and the transposed self features.
    cuts = [c for c in icuts if c < gnc] + [gnc]
    bufspec = []
    p = 0
    for bi, c1 in enumerate(cuts):
        taps = [o for o in _TAPS if caps[o] and p <= offs[o] < c1]
        bufspec.append(dict(c0=p, c1=c1, taps=taps, has_self=False))
        p = c1
    bufspec[-1]["taps"].append(13)
    bufspec[-1]["has_self"] = True
    gchunks = [(b["c0"], b["c1"]) for b in bufspec]

    # matmul segments per psum_block of the edge stream: (block, c0, c1, tap)
    # (c0/c1 are block-local columns; products go to PSUM columns, so no
    # partition-alignment constraints).
    def _clip_spans(spans, total, extra_cuts=()):
        cuts = sorted(set(range(0, total + psum_block, psum_block))
                      | set(extra_cuts))
        nblk = -(-total // psum_block)
        out = [[] for _ in range(nblk)]
        for a, bnd, o in spans:
            p = a
            while p < bnd:
                lim = min(bnd, min(c for c in cuts if c > p))
                blk = p // psum_block
                out[blk].append((p - blk * psum_block, lim - blk * psum_block, o))
                p = lim
        return out

    segs_nc = _clip_spans(
        [(offs[o], offs[o] + caps[o], o) for o in _TAPS if caps[o] > 0], gnc,
        extra_cuts=[c0 for (c0, _c1) in gchunks])
    segs_c = _clip_spans([(0, ROWS, 13)], ROWS)

    ixs = np.concatenate([_wrap16(sdst[cc])[None] for cc in range(NCORES)],
                         axis=0)  # [NCORES, 128, gnc//16]

    # tap -> (buffer id, weight column offset within buffer); buffer widths
    wcol = {}
    for bi, b in enumerate(bufspec):
        for i, o in enumerate(b["taps"]):
            wcol[o] = (bi, i * C)
        b["nw"] = len(b["taps"]) * C
        b["W"] = b["nw"] + (b["c1"] - b["c0"]) + (ROWS if b["has_self"] else 0)

    return dict(gnc=gnc, segs_nc=segs_nc, segs_c=segs_c, gchunks=gchunks,
                uchunks=uchunks, psum_block=psum_block, ixs=ixs, gsrc=gsrc,
                rowmap=rowmap,
                bufspec=bufspec, wcol=wcol, queues=queues, warm_pre=warm_pre,
                bridges=dict(bridges), copy_split=copy_split, copy_eng=copy_eng,
                sig=(gnc, tuple(caps[o] for o in _TAPS), tuple(uchunks_spec),
                     psum_block, tuple(cuts), queues, warm_pre,
                     tuple(sorted(dict(bridges).items())), copy_split, copy_eng))


# ---------------- Bass device program ----------------
_CACHED = {}


def _elide_same_engine_waits(nc):
    """Drop semaphore waits that are implied by in-order engine execution:
    a wait whose semaphore is incremented ONLY by earlier non-DMA
    instructions on the SAME engine is always satisfied by the time the
    instruction executes (engines run their queues in order; only DMA
    completions are asynchronous to the issuing engine). The tile
    framework's per-engine clock sems produce such waits, e.g. the second
    scatter chunk waiting on the first one's Pool clock tick."""
    import concourse.mybir as mybir

    for fn in nc.m.functions:
        for bb in fn.blocks:
            insts = list(bb.instructions)
            sem_updaters = {}  # sem id -> [(engine, engine_pos, inc|None, is_async)]
            engpos = {}
            counter = {}
            for inst in insts:
                e = inst.engine
                p = counter.get(e, 0)
                counter[e] = p + 1
                engpos[inst.name] = p
                si = inst.sync_info
                if si is None:
                    continue
                is_async = "DMA" in type(inst).__name__ or "Dma" in type(inst).__name__
                for u in (si.on_update or []):
                    if u.update_mode == "sem-inc":
                        inc = 1
                    elif u.update_mode == "sem-add-imm":
                        inc = u.update_value
                    else:
                        inc = None  # barrier-style (sub) updates: never elide
                    sem_updaters.setdefault(u.id, []).append((e, p, inc, is_async))
            for inst in insts:
                si = inst.sync_info
                if si is None or not si.on_wait:
                    continue
                e, p = inst.engine, engpos[inst.name]
                keep = []
                for w in si.on_wait:
                    ups = sem_updaters.get(w.id, [])
                    drop = False
                    if w.wait_mode == "sem-ge-imm" and ups:
                        ok = all(ue == e and ui is not None and not ua
                                 for (ue, _up, ui, ua) in ups)
                        if ok:
                            tot = sum(ui for (_ue, up, ui, _ua) in ups if up < p)
                            drop = tot >= w.wait_value
                    if not drop:
                        keep.append(w)
                if len(keep) != len(list(si.on_wait)):
                    inst.sync_info = mybir.SyncInfo(
                        on_wait=keep, on_update=list(si.on_update or []))


def _split_multiwait(nc):
    """Walrus encodes at most one sync wait per instruction. Hoist extra
    waits onto same-engine NOPs inserted just before."""
    import concourse.mybir as mybir

    ctr = 0
    for fn in nc.m.functions:
        for bb in fn.blocks:
            insts = bb.instructions
            orig = list(insts)
            newlist = []
            for inst in orig:
                si = inst.sync_info
                waits = list(si.on_wait or []) if si is not None else []
                if len(waits) >= 2:
                    for w in waits:
                        nop = mybir.InstNoOp(name=f"I-wsplit{ctr}", ins=[], outs=[])
                        ctr += 1
                        nop.engine = inst.engine
                        nop.sync_info = mybir.SyncInfo(on_wait=[w], on_update=[])
                        # register so CoreSim's race detector sees it (its
                        # fake-sem-update pass walks inst_map, not the blocks)
                        nc.inst_map[nop.name] = nop
                        newlist.append(nop)
                    inst.sync_info = mybir.SyncInfo(
                        on_wait=[], on_update=list(si.on_update or []))
                newlist.append(inst)
            insts.clear()
            insts.extend(newlist)


def _build_conv_program(plan):
    import concourse.bass as bass
    import concourse.mybir as mybir
    import concourse.tile as tile
    from concourse import library_config

    nc = bass.Bass("TRN2")
    f32 = mybir.dt.float32
    bf16 = mybir.dt.bfloat16
    i16 = mybir.dt.int16

    gnc = plan["gnc"]
    segs_nc = plan["segs_nc"]
    segs_c = plan["segs_c"]
    uchunks = plan["uchunks"]
    pb = plan["psum_block"]
    wcol = plan["wcol"]
    bufspec = plan["bufspec"]
    queues = plan["queues"]
    warm_pre = plan["warm_pre"]
    bridges = plan["bridges"]
    copy_split = plan["copy_split"]
    copy_eng = plan["copy_eng"]
    ixw = gnc // 16

    bufds = [nc.dram_tensor(f"buf{bi}", [C, b["W"]], bf16, kind="ExternalInput")
             for bi, b in enumerate(bufspec)]
    ixs = nc.dram_tensor("ixs", [128, ixw], i16, kind="ExternalInput")
    Y = nc.dram_tensor("Y", [C, ROWS * 2], bf16, kind="ExternalOutput")
    CEN = nc.dram_tensor("CEN", [C, ROWS], bf16, kind="ExternalOutput")

    from contextlib import ExitStack
    with ExitStack() as ctx:
        tc = ctx.enter_context(
            tile.TileContext(nc, linearize=os.environ.get("KERNEL_LINEARIZE", "0") == "1"))
        const = ctx.enter_context(tc.tile_pool(name="const", bufs=1))
        psum_pool = ctx.enter_context(tc.tile_pool(name="pp", bufs=4, space="PSUM"))
        dmy_pool = ctx.enter_context(tc.tile_pool(name="dmy", bufs=1, space="PSUM"))
        scat_pool = ctx.enter_context(tc.tile_pool(name="scat", bufs=1))

        nc.gpsimd.load_library(library_config.mlp)

        # DMAs in. Only SP and Activation have HWDGE queues; gpsimd can also
        # start (SWDGE) DMAs and is idle until the first scatter. The packed
        # buffers ride the queues given by plan["queues"]; the scatter idxs
        # ride the gpsimd SWDGE queue (they only gate the ucode chain).
        # PE p-state warm-up (optional; off by default — the cost model runs
        # matmuls at 1.2 GHz until 3us of continuous PE busy, but dummy-chain
        # warming measured net-neutral to negative here since the matmuls are
        # off the critical path).
        if warm_pre or bridges:
            dmy = const.tile([C, 64], bf16)
            nc.scalar.memzero(dmy[:])
            pd = dmy_pool.tile([64, 64], f32)

        def emit_warm(n):
            for _ in range(n):
                nc.tensor.matmul(pd[0:64, :], lhsT=dmy[0:C, 0:64],
                                 rhs=dmy[0:C, 0:64],
                                 start=True, stop=True, skip_group_check=True)

        # accumulator [C, ROWS+8, 2] bf16 (j=0 real, j=1 dead lane for d=2,
        # rows >= ROWS take the pad-slot adds), zeroed up front so the
        # scatter chain is independent of the center/self path. The j=1
        # lanes and dump rows accumulate garbage; the host only reads j=0 of
        # rows < ROWS. scat j=1 lanes are never written (uninitialized),
        # matching the accumulator's dead lanes.
        ysb = scat_pool.tile([C, ROWS + 8, 2], bf16)
        nc.vector.memset(ysb[:], 0.0)
        scat = scat_pool.tile([C, gnc, 2], bf16)
        cen = scat_pool.tile([C, ROWS], bf16)

        # A small gpsimd scratch memset before the idx DMA pushes its SWDGE
        # descriptor generation (and so its transfer-ready time) behind the
        # second edge buffer's slot on the globally shared DMA engines; the
        # idx tile still lands well before the first scatter needs it.
        pscr = const.tile([C, 256], bf16)
        nc.gpsimd.memset(pscr[:], 0.0)
        ixb = const.tile([128, ixw], i16)
        nc.gpsimd.dma_start(ixb[:], ixs[:])
        qmap = {"S": nc.sync, "A": nc.scalar}
        bufs = []
        for bi, b in enumerate(bufspec):
            t = const.tile([C, b["W"]], bf16, tag=f"buf{bi}")
            qmap[queues[bi]].dma_start(t[:], bufds[bi][:])
            bufs.append(t)

        emit_warm(warm_pre)

        def wslice(o):
            bi, off = wcol[o]
            return bufs[bi][0:C, off:off + C]

        def rhs_slice(col, ln):
            for bi, b in enumerate(bufspec):
                if b["c0"] <= col < b["c1"]:
                    off = b["nw"] + col - b["c0"]
                    return bufs[bi][0:C, off:off + ln]
            raise AssertionError(col)

        def self_slice(col, ln):
            b = bufspec[-1]
            off = b["nw"] + (b["c1"] - b["c0"])
            return bufs[-1][0:C, off + col:off + col + ln]

        # products: out[oc, edge] = sum_ic W[ic, o, oc] * feat[ic, edge]
        # (weights stationary, feature columns moving -> arbitrary column
        # spans, no PSUM partition-alignment constraints)
        ncopies = 0

        def emit_copy(eng, dst, src):
            if eng is nc.scalar:
                eng.activation(dst, src, mybir.ActivationFunctionType.Copy)
            else:
                eng.tensor_copy(dst, src)

        def emit_block(blk, seglist, lhs_fn, out_tile):
            nonlocal ncopies
            lo = blk * pb
            width = max(c1 for (c0, c1, _o) in seglist)
            pp = psum_pool.tile([C, pb], f32, tag="pp")
            for (c0, c1, o) in seglist:
                nc.tensor.matmul(
                    pp[0:C, c0:c1],
                    lhsT=wslice(o),
                    rhs=lhs_fn(lo + c0, c1 - c0),
                    start=True, stop=True, skip_group_check=True)
            # copy psum -> bf16, split into pieces across copy engines so
            # the scatter's gate closes sooner. In "dvp" mode the blocks of
            # the FIRST scatter chunk split DVE+gpsimd: the gpsimd halves
            # precede the first scatter in its own engine order (no extra
            # semaphore hop), and gpsimd is idle until then anyway. Later
            # blocks stay off gpsimd so they don't wedge between scatters.
            dvp = (copy_eng == "dvp" and out_tile is scat
                   and lo + width <= uchunks[0][1])
            pieces = 2 if dvp else max(1, min(copy_split, width // 128))
            step = -(-width // pieces)
            for pi in range(pieces):
                a, bnd = pi * step, min((pi + 1) * step, width)
                if dvp:
                    eng = nc.vector if pi == 0 else nc.gpsimd
                elif copy_eng in ("dve", "dvp"):
                    eng = nc.vector
                elif copy_eng == "act":
                    eng = nc.scalar
                else:
                    eng = nc.vector if (ncopies % 2 == 0) else nc.scalar
                if out_tile is cen:
                    dst = out_tile[:, lo + a:lo + bnd]
                else:
                    dst = out_tile[:, lo + a:lo + bnd, 0]
                emit_copy(eng, dst, pp[0:C, a:bnd])
                ncopies += 1

        def ucode_of(col):
            for ui, (u0, u1) in enumerate(uchunks):
                if u0 <= col < u1:
                    return ui
            raise AssertionError(col)

        # non-center blocks fill the ucode add stream; alternate copy
        # engines within each ucode chunk
        emitted_isa = set()
        for blk, seglist in enumerate(segs_nc):
            emit_block(blk, seglist, rhs_slice, scat)
            # keep the PE hot across data-arrival gaps
            emit_warm(bridges.get(blk, 0))
            # emit each scatter chunk as soon as its blocks are done so the
            # Pool SEQ order matches data availability
            nxt = (blk + 1) * pb
            for ui, (u0, u1) in enumerate(uchunks):
                if ui not in emitted_isa and nxt >= u1:
                    nc.gpsimd.scatter_add(
                        ysb[:], ixb[0:C, u0 // 16:u1 // 16],
                        scat[:, u0:u1, :], C, ROWS + 8, 2, u1 - u0)
                    emitted_isa.add(ui)
        assert emitted_isa == set(range(len(uchunks)))

        # center blocks go to their own tile (host adds it to the scatter
        # accumulator), keeping the self path off the ucode chain
        for blk, seglist in enumerate(segs_c):
            emit_block(blk, seglist, self_slice, cen)

        nc.scalar.dma_start(CEN[:], cen[:])
        nc.sync.dma_start(Y[:], ysb[:, 0:ROWS, :])
    if os.environ.get("KERNEL_ELIDE_WAITS", "1") == "1":
        _elide_same_engine_waits(nc)
    if os.environ.get("KERNEL_SPLIT_MULTIWAIT", "1") == "1":
        _split_multiwait(nc)
    # Raw Bass skips Bacc's codegen_inst_isa_subclasses pass; without it the
    # NEFF compiler sees empty .instr bytes for extended-ISA instructions
    # (e.g. the library reload) and fails with "ISA wrong length".
    mybir.codegen_inst_isa_subclasses(nc)
    return nc


def _run_conv(feats_f32, plan, w_flat):
    """feats_f32 [N, C] f32, w_flat [27, C, C] f32 -> conv output [N, C] f32
    (no bias; SubMConv3d has none)."""
    from concourse.bass_utils import run_bass_kernel_spmd

    if _CACHED.get("sig") != plan["sig"]:
        _CACHED["nc"] = _build_conv_program(plan)
        _CACHED["sig"] = plan["sig"]
    nc = _CACHED["nc"]

    import ml_dtypes
    fpb = np.ascontiguousarray(feats_f32).astype(ml_dtypes.bfloat16)
    wb = np.ascontiguousarray(np.transpose(w_flat, (1, 0, 2)))  # [ic, o, oc]
    wbb = wb.astype(ml_dtypes.bfloat16)

    gnc = plan["gnc"]
    gsrc = plan["gsrc"]
    in_maps = []
    for cc in range(NCORES):
        edgeT = np.zeros((C, gnc), dtype=ml_dtypes.bfloat16)
        real = gsrc[cc] < N
        edgeT[:, real] = fpb[gsrc[cc][real]].T
        m = {"ixs": plan["ixs"][cc]}
        for bi, b in enumerate(plan["bufspec"]):
            parts = [wbb[:, o, :] for o in b["taps"]]
            parts.append(edgeT[:, b["c0"]:b["c1"]])
            if b["has_self"]:
                parts.append(fpb[plan["rowmap"][cc], :].T)
            buf = np.concatenate(parts, axis=1)
            assert buf.shape[1] == b["W"]
            m[f"buf{bi}"] = np.ascontiguousarray(buf)
        in_maps.append(m)
    trace = os.environ.get("KERNEL_TRACE", "") == "1"
    res = run_bass_kernel_spmd(nc, in_maps, core_ids=list(range(NCORES)), trace=trace)
    if trace and res.exec_time_ns is not None:
        print(f"HW exec time: {res.exec_time_ns} ns")
        _CACHED.setdefault("exec_ns", []).append(res.exec_time_ns)
    out = np.empty((N, C), dtype=np.float32)
    for cc in range(NCORES):
        Yc = np.asarray(res.results[cc]["Y"]).astype(np.float32).reshape(C, ROWS, 2)
        Cc = np.asarray(res.results[cc]["CEN"]).astype(np.float32)
        out[plan["rowmap"][cc]] = (Yc[:, :, 0] + Cc).T
    return out


def _conv_host(feats_f32, plan, w_flat):
    """Host fallback/validation path for the conv (numpy, fp32)."""
    del plan
    gidx = _build_gather(_CACHED["indices"])
    acc = np.zeros((N, C), dtype=np.float32)
    for o in range(27):
        v = gidx[:, o] >= 0
        acc[v] += feats_f32[gidx[v, o]] @ w_flat[o]
    return acc


def kernel(**inputs):
    inputs = {k: np.asarray(v) for k, v in inputs.items()}
    fused = _host_pre(
        inputs['x'], inputs['indices'], inputs['fp_w'], inputs['fp_b'], inputs['fp_g'],
        inputs['fp_be'], inputs['att_w1'], inputs['att_b1'], inputs['att_w2'], inputs['att_b2'],
        inputs['ff_w1'], inputs['ff_b1'], inputs['ff_g'], inputs['ff_be'], inputs['ff_w2'],
        inputs['ff_b2'], inputs['sa_w1'], inputs['sa_b1'], inputs['sa_w2'], inputs['sa_b2'],
        inputs['fj_w1'], inputs['fj_b1'], inputs['fj_g'], inputs['fj_be'], inputs['fj_w2'],
        inputs['fj_b2'], inputs['proj_w'], inputs['proj_g'], inputs['proj_be'], inputs['lw_w'],
        inputs['lw_g'], inputs['lw_be'], inputs['w_w'], inputs['adp_w'], inputs['fuse_w'],
        inputs['fuse_g'], inputs['fuse_be'])

    _CACHED["indices"] = inputs['indices']
    key = inputs['indices'].tobytes()
    if _CACHED.get("plan_key") != key:
        _CACHED["plan"] = _build_edge_plan(inputs['indices'])
        _CACHED["plan_key"] = key
    plan = _CACHED["plan"]

    w1 = inputs['conv1_w'].reshape(27, C, C).astype(np.float32)
    w2 = inputs['conv2_w'].reshape(27, C, C).astype(np.float32)

    conv = _conv_host if os.environ.get("KERNEL_HOST_CONV", "") == "1" else _run_conv

    raw1 = conv(fused, plan, w1)
    f1 = _relu(_bn(raw1, inputs['bn1_g'], inputs['bn1_be']))
    raw2 = conv(f1, plan, w2)
    f2 = _bn(raw2, inputs['bn2_g'], inputs['bn2_be'])
    return _relu(f2 + fused).astype(np.float32)
